# revision 20
# baseline (speedup 1.0000x reference)
"""Distributed causal attention kernel for 8 TRN2 NeuronCores.

Sharding: core c -> (batch b = c//2, head-group g = c%2).  Each core
computes attention for its batch over 8 of the 16 heads plus the partial
output projection (row-parallel Wo); the host sums the two partials per
batch and transposes back.

v3: position-hybrid fp8.  Numerics are bf16 for everything that touches
query positions 0-511 (whose outputs average over few keys and dominate
max-error) and fp8e4m3 + DoubleRow matmuls elsewhere:
  - q/k/v projections: seq chunk 0 bf16, chunks 1-3 fp8 (2 k-rows/cycle)
  - PV: diagonal key-blocks bf16 (also carries the causal mask),
    off-diagonal blocks fp8 DoubleRow packing 2 key-blocks per matmul
  - Wo: query stripe 0 bf16, stripes 1-3 fp8 DoubleRow
  - QK stays bf16: its cost is N-bound and the K=64 head pairs already
    run concurrently on the row-tiled (64x128) PE array.
Scales: wq carries SCALE*CQ, wk CK, wv CV, wo8 CO; vaug ones col = 1/CA
so denominators come out pre-divided and at picks up a factor CA.
Output evac rescales by 1/CA (stripe 0) or 1/(CA*CO) (stripes 1-3).

Exp is evaluated on ScalarE in one activation per (pair, block-pair):
st PSUM is [128, 2, 2, SI] (two key blocks x two heads) so each ACT
covers up to 2048 elements, amortising the ~352-cycle ACT overhead.
Causal masking of diagonal blocks runs on GpSimd (otherwise idle).
"""

import os

import numpy as np

import concourse.bass as bass
import concourse.tile as tile
from concourse import bacc, mybir
from concourse.bass import MemorySpace

F32 = mybir.dt.float32
BF16 = mybir.dt.bfloat16
FP8 = mybir.dt.float8e4
AF = mybir.ActivationFunctionType
DR = mybir.MatmulPerfMode.DoubleRow

B, S, DIM, H = 4, 2048, 1024, 16
HD = DIM // H          # 64
SCALE = HD ** -0.5
NCORES = 8
DG = DIM // 2          # 512 head dims per core (8 heads)
NPAIR = 4              # head pairs per core
SI = 512               # si chunk (query positions per attention stripe)
SJ = 128               # sj chunk (key positions per matmul)
NSI = S // SI          # 4
AC = 512               # seq chunk for projections
NAC = S // AC          # 4
KC = DIM // 128        # 8 contraction chunks for bf16 projections
CA = 8.0               # at scale (ones col = 1/CA)
CQ = 512.0             # fp8 weight scales
CK = 64.0
CV = 64.0
CO = 64.0

LAST_RESULTS = None


def _build_core_kernel():
    nc = bacc.Bacc(
        "TRN2", target_bir_lowering=False, debug=False, num_devices=NCORES
    )

    # chunk-0 (positions 0-511) activations, bf16
    xq0 = nc.dram_tensor("xq0", [DIM, AC], BF16, kind="ExternalInput").ap()
    xk0 = nc.dram_tensor("xk0", [DIM, AC], BF16, kind="ExternalInput").ap()
    xv0 = nc.dram_tensor("xv0", [DIM, AC], BF16, kind="ExternalInput").ap()
    # full activations fp8 (only chunks 1-3 are read)
    x8q = nc.dram_tensor("x8q", [DIM, S], FP8, kind="ExternalInput").ap()
    x8k = nc.dram_tensor("x8k", [DIM, S], FP8, kind="ExternalInput").ap()
    x8v = nc.dram_tensor("x8v", [DIM, S], FP8, kind="ExternalInput").ap()
    wq = nc.dram_tensor("wq", [DIM, DG], BF16, kind="ExternalInput").ap()
    wk = nc.dram_tensor("wk", [DIM, DG], BF16, kind="ExternalInput").ap()
    wv = nc.dram_tensor("wv", [DIM, DG], BF16, kind="ExternalInput").ap()
    w8q = nc.dram_tensor("w8q", [DIM, DG], FP8, kind="ExternalInput").ap()
    w8k = nc.dram_tensor("w8k", [DIM, DG], FP8, kind="ExternalInput").ap()
    w8v = nc.dram_tensor("w8v", [DIM, DG], FP8, kind="ExternalInput").ap()
    wo = nc.dram_tensor("wo", [DG, DIM], BF16, kind="ExternalInput").ap()
    wo8 = nc.dram_tensor("wo8", [DG, DIM], FP8, kind="ExternalInput").ap()
    tri = nc.dram_tensor("tri", [128, 128], BF16, kind="ExternalInput").ap()
    ones = nc.dram_tensor("ones128", [128, 64], BF16, kind="ExternalInput").ap()
    out = nc.dram_tensor("out", [DIM, S], F32, kind="ExternalOutput").ap()

    # partition-tiled DRAM views
    xq0_v = xq0.rearrange("(kc p) s -> p kc s", p=128)   # [128, 8, 512]
    xk0_v = xk0.rearrange("(kc p) s -> p kc s", p=128)
    xv0_v = xv0.rearrange("(kc p) s -> p kc s", p=128)
    x8q_v = x8q.rearrange("(kcp two p) s -> p kcp two s", p=128, two=2)
    x8k_v = x8k.rearrange("(kcp two p) s -> p kcp two s", p=128, two=2)
    x8v_v = x8v.rearrange("(kcp two p) s -> p kcp two s", p=128, two=2)
    wq_v = wq.rearrange("(kc p) m -> p kc m", p=128)     # [128, 8, 512]
    wk_v = wk.rearrange("(kc p) m -> p kc m", p=128)
    wv_v = wv.rearrange("(kc p) m -> p kc m", p=128)
    w8q_v = w8q.rearrange("(kcp two p) m -> p kcp two m", p=128, two=2)
    w8k_v = w8k.rearrange("(kcp two p) m -> p kcp two m", p=128, two=2)
    w8v_v = w8v.rearrange("(kcp two p) m -> p kcp two m", p=128, two=2)
    wo_v = wo.rearrange("(kt p) m -> p kt m", p=128)     # [128, 4, 1024]
    wo8_v = wo8.rearrange("(ktp two p) m -> p ktp two m", p=128, two=2)
    out_v = out.rearrange("(mt p) s -> p mt s", p=128)   # [128, 8, 2048]

    with tile.TileContext(nc) as tc:
        with (
            tc.tile_pool(name="persist", bufs=1) as persist,
            tc.tile_pool(name="cw", bufs=1) as cwpool,
            tc.tile_pool(name="co", bufs=2) as copool,
        ):
            # persistent SBUF tensors
            qT = persist.tile([128, NPAIR, S], BF16)        # [64l+d, pair, si]
            kT = persist.tile([128, NPAIR, S], BF16)
            vaug = persist.tile([128, S // SJ, 8, HD + 1], BF16)  # [sj, j, h, d|1/CA]
            vaug8 = persist.tile([128, 6, 2, 8, HD + 2], FP8)     # [sj, jp, jj, h, d|1/CA|pad]
            at = persist.tile([128, NPAIR, S], BF16)        # unnorm A.T * CA-pre
            at8 = persist.tile([128, 2, 2, 3 * SI], FP8)    # normed, stripes 1-3
            rden = persist.tile([65, NPAIR, NSI, 2, SI], BF16)  # raw denom/CA @ p64
            tri_sb = persist.tile([128, 128], BF16)
            ones_sb = persist.tile([128, 64], BF16)
            warm_sb = persist.tile([128, 128], BF16)
            wo_bf = cwpool.tile([128, 4, DIM], BF16, tag="wo16")
            wo8_sb = cwpool.tile([128, 2, 2, DIM], FP8, tag="wo8")

            nc.vector.memset(warm_sb[:], 0.125)
            nc.sync.dma_start(out=tri_sb[:], in_=tri[:, :])
            nc.sync.dma_start(out=ones_sb[:], in_=ones[:, :])
            # ones columns of vaug/vaug8 hold 1/CA so denominators come out
            # pre-divided by CA (normalised at then carries a factor CA).
            nc.vector.memset(vaug[:, :, :, HD], 1.0 / CA)
            nc.vector.memset(vaug8[:, :, :, :, HD], 1.0 / CA)

            def ka():
                # HAM keep-alive: a 32-col dummy weight load (~27ns) resets
                # the PE idle timer so sparse phases don't get clock-gated.
                nc.tensor.ldweights(tri_sb[:, 0:32])

            def norm_unit(i, p, bpsum):
                """Broadcast 1/denominator and write normalised at/at8."""
                ssl = slice(i * SI, (i + 1) * SI)
                bc = bpsum.tile([128, SI], F32, tag="bc", name="bc")
                for l in range(2):
                    nc.tensor.matmul(
                        bc[64 * l:64 * l + 64, :],
                        ones_sb[64:65, 0:64],
                        rden[64:65, p, i, l, :],
                        start=True,
                        stop=True,
                    )
                ka()
                rbc = copool.tile([128, SI], F32, tag="rbc", name="rbc")
                nc.vector.reciprocal_approx_fast(rbc[:, :], bc[:, :])
                if i == 0:
                    nc.vector.tensor_mul(at[:, p, ssl], at[:, p, ssl], rbc[:, :])
                else:
                    nc.vector.tensor_mul(
                        at8[:, p // 2, p % 2, (i - 1) * SI:i * SI],
                        at[:, p, ssl],
                        rbc[:, :],
                    )

            def wo_chain(i, mt, cpsum):
                """Output projection for one 128-row block of out, stripe i."""
                ssl = slice(i * SI, (i + 1) * SI)
                ps = cpsum.tile([128, SI], F32, tag="cps", name="cps")
                if i == 0:
                    for kt in range(4):
                        nc.tensor.matmul(
                            ps[:, :],
                            wo_bf[:, kt, mt * 128:(mt + 1) * 128],
                            at[:, kt, ssl],
                            start=(kt == 0),
                            stop=(kt == 3),
                        )
                    oscale = 1.0 / CA
                else:
                    a8sl = slice((i - 1) * SI, i * SI)
                    for ktp in range(2):
                        nc.tensor.matmul(
                            ps[:, :],
                            wo8_sb[:, ktp, :, mt * 128:(mt + 1) * 128],
                            at8[:, ktp, :, a8sl],
                            start=(ktp == 0),
                            stop=(ktp == 1),
                            perf_mode=DR,
                        )
                    oscale = 1.0 / (CA * CO)
                osb = copool.tile([128, SI], F32, tag="osb", name="osb")
                nc.vector.tensor_scalar_mul(osb[:, :], ps[:, :], oscale)
                nc.sync.dma_start(out=out_v[:, mt, ssl], in_=osb[:, :])
                ka()

            with (
                tc.tile_pool(name="ptb", bufs=2) as ptbpool,
                tc.tile_pool(name="pt8", bufs=2) as pt8pool,
                tc.tile_pool(name="stps", bufs=2, space=MemorySpace.PSUM) as stps,
                tc.tile_pool(name="ops", bufs=2, space=MemorySpace.PSUM) as ops,
            ):

                def make_stripe(i, pair_done=None):
                    """Emission units for attention stripe i (all pairs).

                    Unit = one key-block-pair jp: 4 QK matmuls (row-tiled
                    concurrent head pairs), one big exp ACT, causal masks on
                    GpSimd for diagonal blocks, then PV (fp8 DoubleRow for
                    off-diagonal jp, bf16 for the diagonal ones).
                    """
                    si0 = i * SI
                    ssl = slice(si0, si0 + SI)
                    njs = 4 * i + 4
                    units = []
                    for p in range(NPAIR):
                        state = {}

                        def start_pair(p=p, state=state):
                            state["oa"] = ops.tile(
                                [65, SI], F32, tag="o2", name="oa"
                            )
                            state["ob"] = ops.tile(
                                [65, SI], F32, tag="o2", name="ob"
                            )

                        def unit2(j0, p=p, state=state, i=i, si0=si0):
                            """One emission unit = key blocks (j0, j0+1):
                            both QK pairs back-to-back (one 64-mode window),
                            both exps, masks, then the PV matmuls."""
                            njs = 4 * i + 4
                            nondiag = j0 < 4 * i
                            sts, pts, r0s = [], [], []
                            for j in (j0, j0 + 1):
                                d0 = j * SJ - si0
                                r0 = max(0, d0)
                                r0s.append(r0)
                                st = stps.tile(
                                    [128, 2, SI], F32, tag="st", name="st"
                                )
                                sts.append(st)
                                for l in range(2):
                                    lsl = slice(64 * l, 64 * l + 64)
                                    nc.tensor.matmul(
                                        st[:, l, r0:SI],
                                        kT[lsl, p, j * SJ:(j + 1) * SJ],
                                        qT[lsl, p, si0 + r0:si0 + SI],
                                        start=True,
                                        stop=True,
                                    )
                            if nondiag:
                                pt8 = pt8pool.tile(
                                    [128, 2, 2, SI], FP8, tag="pt8", name="pt8"
                                )
                                for jj in range(2):
                                    nc.scalar.activation(
                                        pt8[:, jj, :, :], sts[jj][:, :, :], AF.Exp
                                    )
                                for l in range(2):
                                    nc.tensor.matmul(
                                        (state["oa"] if l == 0 else state["ob"])[:, :],
                                        vaug8[:, j0 // 2, :, 2 * p + l, 0:HD + 1],
                                        pt8[:, :, l, :],
                                        start=(j0 == 0),
                                        stop=False,
                                        perf_mode=DR,
                                        skip_group_check=True,
                                    )
                            else:
                                for jj in range(2):
                                    j = j0 + jj
                                    r0 = r0s[jj]
                                    d0 = j * SJ - si0
                                    pt = ptbpool.tile(
                                        [128, 2, SI], BF16, tag="ptb", name="ptb"
                                    )
                                    pts.append(pt)
                                    nc.scalar.activation(
                                        pt[:, :, r0:SI], sts[jj][:, :, r0:SI], AF.Exp
                                    )
                                    if d0 >= 0:
                                        for l in range(2):
                                            nc.vector.tensor_mul(
                                                pt[:, l, d0:d0 + 128],
                                                pt[:, l, d0:d0 + 128],
                                                tri_sb[:, :],
                                            )
                                for jj in range(2):
                                    j = j0 + jj
                                    r0 = r0s[jj]
                                    for l in range(2):
                                        nc.tensor.matmul(
                                            (state["oa"] if l == 0 else state["ob"])[:, r0:SI],
                                            vaug[:, j, 2 * p + l, :],
                                            pts[jj][:, l, r0:SI],
                                            start=(j == 0),
                                            stop=(j == njs - 1),
                                            skip_group_check=True,
                                        )

                        def unit2_ka(j0, u=None, i=i):
                            u(j0)
                            npulse = 2 if i < 2 else 4
                            for _ in range(npulse):
                                ka()

                        def end_pair(p=p, state=state, ssl=ssl, i=i):
                            ka()
                            for l in range(2):
                                o2 = state["oa"] if l == 0 else state["ob"]
                                nc.vector.tensor_copy(
                                    at[64 * l:64 * l + 64, p, ssl], o2[0:HD, :]
                                )
                                nc.vector.tensor_copy(
                                    rden[64:65, p, i, l, :], o2[HD:HD + 1, :]
                                )
                            ka()

                        units.append(start_pair)
                        for j0 in range(0, njs, 2):
                            units.append(lambda j0=j0, u=unit2: unit2_ka(j0, u))
                        units.append(end_pair)
                        if pair_done is not None:
                            units.append(lambda p=p: pair_done(p))
                    return units

                def emit_interleaved(units, fillers):
                    """Emit units with fillers distributed evenly between."""
                    U, F = len(units), len(fillers)
                    fi = 0
                    for k, u in enumerate(units):
                        u()
                        want = (k + 1) * F // U
                        while fi < want:
                            fillers[fi]()
                            fi += 1
                    while fi < F:
                        fillers[fi]()
                        fi += 1

                with (
                    tc.tile_pool(name="ax", bufs=3) as xpool,
                    tc.tile_pool(name="ax0", bufs=2) as x0pool,
                    tc.tile_pool(name="aw", bufs=1) as wpool,
                    tc.tile_pool(name="aps", bufs=1, space=MemorySpace.PSUM) as apsum,
                ):
                    wq_sb = wpool.tile([128, KC, DG], BF16, tag="w0")
                    wk_sb = wpool.tile([128, KC, DG], BF16, tag="wk")
                    wv_sb = wpool.tile([128, KC, DG], BF16, tag="w0")
                    w8q_sb = wpool.tile([128, 4, 2, DG], FP8, tag="w8q")
                    w8k_sb = wpool.tile([128, 4, 2, DG], FP8, tag="w8k")
                    w8v_sb = wpool.tile([128, 4, 2, DG], FP8, tag="w8v")
                    w8_sb = {"q": w8q_sb, "k": w8k_sb, "v": w8v_sb}
                    x8_view = {"q": x8q_v, "k": x8k_v, "v": x8v_v}
                    x8_tiles = {}
                    pscale = {"q": 1.0 / CQ, "k": 1.0 / CK, "v": 1.0 / CV}

                    def dma_x8(t, n):
                        xt = xpool.tile(
                            [128, 4, 2, AC], FP8, tag="x8", name=f"x8_{t}{n}"
                        )
                        nc.sync.dma_start(
                            out=xt[:], in_=x8_view[t][:, :, :, n * AC:(n + 1) * AC]
                        )
                        x8_tiles[(t, n)] = xt

                    def chain_qk8(t, n, p):
                        """fp8 DoubleRow q/k projection chain, chunks 1-3."""
                        sl = slice(n * AC, (n + 1) * AC)
                        xt = x8_tiles[(t, n)]
                        ps = apsum.tile([128, AC], F32, tag="aps", name="aps")
                        for kcp in range(4):
                            nc.tensor.matmul(
                                ps[:, :],
                                w8_sb[t][:, kcp, :, p * 128:(p + 1) * 128],
                                xt[:, kcp, :, :],
                                start=(kcp == 0),
                                stop=(kcp == 3),
                                perf_mode=DR,
                            )
                        nc.vector.tensor_scalar_mul(
                            (qT if t == "q" else kT)[:, p, sl], ps[:, :], pscale[t]
                        )

                    def chain_v8(n, mm):
                        """fp8 DoubleRow v projection, one 128-seq block."""
                        xt = x8_tiles[("v", n)]
                        j = n * (AC // 128) + mm
                        ps = apsum.tile([128, DG], F32, tag="aps", name="apsv")
                        for kcp in range(4):
                            nc.tensor.matmul(
                                ps[:, :],
                                xt[:, kcp, :, mm * 128:(mm + 1) * 128],
                                w8_sb["v"][:, kcp, :, :],
                                start=(kcp == 0),
                                stop=(kcp == 3),
                                perf_mode=DR,
                            )
                        nc.vector.tensor_scalar_mul(
                            vaug[:, j, :, 0:HD], ps[:, :], pscale["v"]
                        )
                        if j < 12:
                            nc.vector.tensor_scalar_mul(
                                vaug8[:, j // 2, j % 2, :, 0:HD],
                                ps[:, :],
                                pscale["v"],
                            )

                    def chunk_fillers(n):
                        fs = [lambda t=t, n=n: dma_x8(t, n) for t in ("q", "k", "v")]
                        for p in range(NPAIR):
                            fs.append(lambda p=p, n=n: chain_qk8("q", n, p))
                        for p in range(NPAIR):
                            fs.append(lambda p=p, n=n: chain_qk8("k", n, p))
                        for mm in range(AC // 128):
                            fs.append(lambda mm=mm, n=n: chain_v8(n, mm))
                        return fs

                    # chunk 0: stream the q projection kc-major behind
                    # per-kc DMA slices so the first matmul fires after the
                    # first 128KB lands instead of the full 0.5MB.
                    xt0 = x0pool.tile([128, KC, AC], BF16, tag="x0", name="x_q0")
                    for kc in range(KC):
                        nc.sync.dma_start(
                            out=wq_sb[:, kc, :], in_=wq_v[:, kc, :]
                        )
                        nc.sync.dma_start(
                            out=xt0[:, kc, :], in_=xq0_v[:, kc, :]
                        )
                    # HAM warmup: ~4.5us of throwaway matmuls gated on the
                    # tiny tri/ones DMAs so they run while the first input
                    # slices land and hand over to the q0 chains with no gap.
                    warm = apsum.tile([64, 128], F32, tag="aps", name="warm")
                    for _ in range(44):
                        nc.tensor.matmul(
                            warm[:, :], ones_sb[:, :], tri_sb[:, :],
                            start=True, stop=True,
                        )
                    def q0_phase(ph):
                        pss = [
                            apsum.tile([128, AC], F32, tag="q0ps", name="q0ps")
                            for _ in range(2)
                        ]
                        for kc in range(KC):
                            for pi in range(2):
                                p = 2 * ph + pi
                                nc.tensor.matmul(
                                    pss[pi][:, :],
                                    wq_sb[:, kc, p * 128:(p + 1) * 128],
                                    xt0[:, kc, :],
                                    start=(kc == 0),
                                    stop=(kc == KC - 1),
                                )
                        for pi in range(2):
                            nc.vector.tensor_copy(
                                qT[:, 2 * ph + pi, 0:AC], pss[pi][:, :]
                            )

                    q0_phase(0)
                    # chunk-0 k and v (bf16)
                    xtk0 = x0pool.tile([128, KC, AC], BF16, tag="x0", name="x_k0")
                    nc.sync.dma_start(out=xtk0[:], in_=xk0_v[:, :, :])
                    nc.sync.dma_start(out=wk_sb[:], in_=wk_v[:, :, :])
                    nc.sync.dma_start(out=wv_sb[:], in_=wv_v[:, :, :])
                    xtv0 = x0pool.tile([128, KC, AC], BF16, tag="x0", name="x_v0")
                    nc.sync.dma_start(out=xtv0[:], in_=xv0_v[:, :, :])
                    nc.sync.dma_start(out=w8q_sb[:], in_=w8q_v[:, :, :, :])
                    nc.sync.dma_start(out=w8k_sb[:], in_=w8k_v[:, :, :, :])
                    nc.sync.dma_start(out=w8v_sb[:], in_=w8v_v[:, :, :, :])

                    def chain_k0(p):
                        ps = apsum.tile([128, AC], F32, tag="aps", name="apsk0")
                        for kc in range(KC):
                            nc.tensor.matmul(
                                ps[:, :],
                                wk_sb[:, kc, p * 128:(p + 1) * 128],
                                xtk0[:, kc, :],
                                start=(kc == 0),
                                stop=(kc == KC - 1),
                            )
                        nc.vector.tensor_copy(kT[:, p, 0:AC], ps[:, :])

                    def chain_v0(mm):
                        ps = apsum.tile([128, DG], F32, tag="aps", name="apsv0")
                        for kc in range(KC):
                            nc.tensor.matmul(
                                ps[:, :],
                                xtv0[:, kc, mm * 128:(mm + 1) * 128],
                                wv_sb[:, kc, :],
                                start=(kc == 0),
                                stop=(kc == KC - 1),
                            )
                        nc.vector.tensor_copy(vaug[:, mm, :, 0:HD], ps[:, :])
                        nc.vector.tensor_copy(
                            vaug8[:, mm // 2, mm % 2, :, 0:HD], ps[:, :]
                        )

                    for p in range(2):
                        chain_k0(p)
                    for mm in range(AC // 128):
                        chain_v0(mm)

                    s0_fillers = [
                        lambda: q0_phase(1),
                        lambda: chain_k0(2),
                        lambda: chain_k0(3),
                    ] + chunk_fillers(1)
                    emit_interleaved(make_stripe(0), s0_fillers)
                    emit_interleaved(make_stripe(1), chunk_fillers(2))
                    emit_interleaved(make_stripe(2), chunk_fillers(3))

                # ---- last stripe interleaves with normalise + Wo (1-buf
                # ---- psum pools; stalls absorb into exp waits)
                with (
                    tc.tile_pool(name="cps1", bufs=1, space=MemorySpace.PSUM) as cps1,
                    tc.tile_pool(name="bps1", bufs=1, space=MemorySpace.PSUM) as bps1,
                ):
                    c_fillers = [
                        lambda: nc.sync.dma_start(out=wo_bf[:], in_=wo_v[:, :, :]),
                        lambda: nc.sync.dma_start(out=wo8_sb[:], in_=wo8_v[:, :, :, :]),
                    ]
                    for i in range(3):
                        for p in range(NPAIR):
                            c_fillers.append(
                                lambda i=i, p=p: norm_unit(i, p, bps1)
                            )
                        for mt in range(8):
                            c_fillers.append(
                                lambda i=i, mt=mt: wo_chain(i, mt, cps1)
                            )

                    emit_interleaved(
                        make_stripe(3, pair_done=lambda p: norm_unit(3, p, bps1)),
                        c_fillers,
                    )

            # final output stripe with room to pipeline (st/o2 closed)
            with (
                tc.tile_pool(name="cps2", bufs=4, space=MemorySpace.PSUM) as cps2,
            ):
                for mt in range(8):
                    wo_chain(3, mt, cps2)

    nc.compile()
    return nc


_NC_CACHE = {}


def _get_nc():
    if "nc" not in _NC_CACHE:
        _NC_CACHE["nc"] = _build_core_kernel()
    return _NC_CACHE["nc"]


def make_in_maps(query, key, value, Wq, Wk, Wv, Wo):
    import ml_dtypes

    bf = ml_dtypes.bfloat16
    f8 = ml_dtypes.float8_e4m3
    tri = (np.arange(128)[None, :] >= np.arange(128)[:, None]).astype(bf)
    ones128 = np.ones((128, 64), bf)
    # per-batch transposed activations (shared by the batch's two cores):
    # chunk 0 in bf16, the full sequence in fp8
    xq0_b, xk0_b, xv0_b, x8q_b, x8k_b, x8v_b = [], [], [], [], [], []
    for b in range(B):
        for src, lo, hi in ((query, xq0_b, x8q_b), (key, xk0_b, x8k_b),
                            (value, xv0_b, x8v_b)):
            xt = np.ascontiguousarray(src[b].T)
            lo.append(xt[:, :AC].astype(bf))
            hi.append(xt.astype(f8))
    w_g = []
    for g in range(2):
        rows = slice(g * DG, (g + 1) * DG)
        wq_g = Wq[rows, :] * SCALE
        w_g.append({
            "wq": np.ascontiguousarray(wq_g.T).astype(bf),
            "wk": np.ascontiguousarray(Wk[rows, :].T).astype(bf),
            "wv": np.ascontiguousarray(Wv[rows, :].T).astype(bf),
            "w8q": np.ascontiguousarray((wq_g * CQ).T).astype(f8),
            "w8k": np.ascontiguousarray((Wk[rows, :] * CK).T).astype(f8),
            "w8v": np.ascontiguousarray((Wv[rows, :] * CV).T).astype(f8),
            "wo": np.ascontiguousarray(Wo[:, rows].T).astype(bf),
            "wo8": np.ascontiguousarray((Wo[:, rows] * CO).T).astype(f8),
        })
    in_maps = []
    for c in range(NCORES):
        b, g = c // 2, c % 2
        in_maps.append({
            "xq0": xq0_b[b], "xk0": xk0_b[b], "xv0": xv0_b[b],
            "x8q": x8q_b[b], "x8k": x8k_b[b], "x8v": x8v_b[b],
            **w_g[g], "tri": tri, "ones128": ones128,
        })
    return in_maps


def kernel(query, key, value, attn_mask, Wq, Wk, Wv, Wo):
    global LAST_RESULTS
    from concourse.bass_utils import run_bass_kernel_spmd

    query = np.asarray(query, np.float32)
    key = np.asarray(key, np.float32)
    value = np.asarray(value, np.float32)
    Wq = np.asarray(Wq, np.float32)
    Wk = np.asarray(Wk, np.float32)
    Wv = np.asarray(Wv, np.float32)
    Wo = np.asarray(Wo, np.float32)

    nc = _get_nc()
    in_maps = make_in_maps(query, key, value, Wq, Wk, Wv, Wo)
    res = run_bass_kernel_spmd(
        nc,
        in_maps,
        core_ids=list(range(NCORES)),
        trace=bool(int(os.environ.get("KERNEL_TRACE", "0"))),
    )
    LAST_RESULTS = res

    full = np.empty((B, S, DIM), np.float32)
    for b in range(B):
        full[b] = (res.results[2 * b]["out"] + res.results[2 * b + 1]["out"]).T
    return full


# revision 21
# speedup vs baseline: 1.0272x; 1.0272x over previous
"""Distributed causal attention kernel for 8 TRN2 NeuronCores.

Sharding: core c -> (batch b = c//2, head-group g = c%2).  Each core
computes attention for its batch over 8 of the 16 heads plus the partial
output projection (row-parallel Wo); the host sums the two partials per
batch and transposes back.

v3: position-hybrid fp8.  Numerics are bf16 for everything that touches
query positions 0-511 (whose outputs average over few keys and dominate
max-error) and fp8e4m3 + DoubleRow matmuls elsewhere:
  - q/k/v projections: seq chunk 0 bf16, chunks 1-3 fp8 (2 k-rows/cycle)
  - PV: diagonal key-blocks bf16 (also carries the causal mask),
    off-diagonal blocks fp8 DoubleRow packing 2 key-blocks per matmul
  - Wo: query stripe 0 bf16, stripes 1-3 fp8 DoubleRow
  - QK stays bf16: its cost is N-bound and the K=64 head pairs already
    run concurrently on the row-tiled (64x128) PE array.
Scales: wq carries SCALE*CQ, wk CK, wv CV, wo8 CO; vaug ones col = 1/CA
so denominators come out pre-divided and at picks up a factor CA.
Output evac rescales by 1/CA (stripe 0) or 1/(CA*CO) (stripes 1-3).

Exp is evaluated on ScalarE in one activation per (pair, block-pair):
st PSUM is [128, 2, 2, SI] (two key blocks x two heads) so each ACT
covers up to 2048 elements, amortising the ~352-cycle ACT overhead.
Causal masking of diagonal blocks runs on GpSimd (otherwise idle).
"""

import os

import numpy as np

import concourse.bass as bass
import concourse.tile as tile
from concourse import bacc, mybir
from concourse.bass import MemorySpace

F32 = mybir.dt.float32
BF16 = mybir.dt.bfloat16
FP8 = mybir.dt.float8e4
AF = mybir.ActivationFunctionType
DR = mybir.MatmulPerfMode.DoubleRow

B, S, DIM, H = 4, 2048, 1024, 16
HD = DIM // H          # 64
SCALE = HD ** -0.5
NCORES = 8
DG = DIM // 2          # 512 head dims per core (8 heads)
NPAIR = 4              # head pairs per core
SI = 512               # si chunk (query positions per attention stripe)
SJ = 128               # sj chunk (key positions per matmul)
NSI = S // SI          # 4
AC = 512               # seq chunk for projections
NAC = S // AC          # 4
KC = DIM // 128        # 8 contraction chunks for bf16 projections
CA = 8.0               # at scale (ones col = 1/CA)
CQ = 512.0             # fp8 weight scales
CK = 64.0
CV = 64.0
CO = 64.0

LAST_RESULTS = None


def _build_core_kernel():
    nc = bacc.Bacc(
        "TRN2", target_bir_lowering=False, debug=False, num_devices=NCORES
    )

    # chunk-0 (positions 0-511) activations, bf16
    xq0 = nc.dram_tensor("xq0", [DIM, AC], BF16, kind="ExternalInput").ap()
    xk0 = nc.dram_tensor("xk0", [DIM, AC], BF16, kind="ExternalInput").ap()
    xv0 = nc.dram_tensor("xv0", [DIM, AC], BF16, kind="ExternalInput").ap()
    # full activations fp8 (only chunks 1-3 are read)
    x8q = nc.dram_tensor("x8q", [DIM, S], FP8, kind="ExternalInput").ap()
    x8k = nc.dram_tensor("x8k", [DIM, S], FP8, kind="ExternalInput").ap()
    x8v = nc.dram_tensor("x8v", [DIM, S], FP8, kind="ExternalInput").ap()
    wq = nc.dram_tensor("wq", [DIM, DG], BF16, kind="ExternalInput").ap()
    wk = nc.dram_tensor("wk", [DIM, DG], BF16, kind="ExternalInput").ap()
    wv = nc.dram_tensor("wv", [DIM, DG], BF16, kind="ExternalInput").ap()
    w8q = nc.dram_tensor("w8q", [DIM, DG], FP8, kind="ExternalInput").ap()
    w8k = nc.dram_tensor("w8k", [DIM, DG], FP8, kind="ExternalInput").ap()
    w8v = nc.dram_tensor("w8v", [DIM, DG], FP8, kind="ExternalInput").ap()
    wo = nc.dram_tensor("wo", [DG, DIM], BF16, kind="ExternalInput").ap()
    wo8 = nc.dram_tensor("wo8", [DG, DIM], FP8, kind="ExternalInput").ap()
    tri = nc.dram_tensor("tri", [128, 128], BF16, kind="ExternalInput").ap()
    ones = nc.dram_tensor("ones128", [128, 64], BF16, kind="ExternalInput").ap()
    out = nc.dram_tensor("out", [DIM, S], F32, kind="ExternalOutput").ap()

    # partition-tiled DRAM views
    xq0_v = xq0.rearrange("(kc p) s -> p kc s", p=128)   # [128, 8, 512]
    xk0_v = xk0.rearrange("(kc p) s -> p kc s", p=128)
    xv0_v = xv0.rearrange("(kc p) s -> p kc s", p=128)
    x8q_v = x8q.rearrange("(kcp two p) s -> p kcp two s", p=128, two=2)
    x8k_v = x8k.rearrange("(kcp two p) s -> p kcp two s", p=128, two=2)
    x8v_v = x8v.rearrange("(kcp two p) s -> p kcp two s", p=128, two=2)
    wq_v = wq.rearrange("(kc p) m -> p kc m", p=128)     # [128, 8, 512]
    wk_v = wk.rearrange("(kc p) m -> p kc m", p=128)
    wv_v = wv.rearrange("(kc p) m -> p kc m", p=128)
    w8q_v = w8q.rearrange("(kcp two p) m -> p kcp two m", p=128, two=2)
    w8k_v = w8k.rearrange("(kcp two p) m -> p kcp two m", p=128, two=2)
    w8v_v = w8v.rearrange("(kcp two p) m -> p kcp two m", p=128, two=2)
    wo_v = wo.rearrange("(kt p) m -> p kt m", p=128)     # [128, 4, 1024]
    wo8_v = wo8.rearrange("(ktp two p) m -> p ktp two m", p=128, two=2)
    out_v = out.rearrange("(mt p) s -> p mt s", p=128)   # [128, 8, 2048]

    with tile.TileContext(nc) as tc:
        with (
            tc.tile_pool(name="persist", bufs=1) as persist,
            tc.tile_pool(name="cw", bufs=1) as cwpool,
            tc.tile_pool(name="co", bufs=2) as copool,
        ):
            # persistent SBUF tensors
            qT = persist.tile([128, NPAIR, S], BF16)        # [64l+d, pair, si]
            kT = persist.tile([128, NPAIR, S], BF16)
            vaug = persist.tile([128, S // SJ, 8, HD + 1], BF16)  # [sj, j, h, d|1/CA]
            vaug8 = persist.tile([128, 6, 2, 8, HD + 2], FP8)     # [sj, jp, jj, h, d|1/CA|pad]
            at = persist.tile([128, NPAIR, S], BF16)        # unnorm A.T * CA-pre
            at8 = persist.tile([128, 2, 2, 3 * SI], FP8)    # normed, stripes 1-3
            rden = persist.tile([65, NPAIR, NSI, 2, SI], BF16)  # raw denom/CA @ p64
            tri_sb = persist.tile([128, 128], BF16)
            ones_sb = persist.tile([128, 64], BF16)
            warm_sb = persist.tile([128, 128], BF16)
            wo_bf = cwpool.tile([128, 4, DIM], BF16, tag="wo16")
            wo8_sb = cwpool.tile([128, 2, 2, DIM], FP8, tag="wo8")

            nc.vector.memset(warm_sb[:], 0.125)
            nc.sync.dma_start(out=tri_sb[:], in_=tri[:, :])
            nc.sync.dma_start(out=ones_sb[:], in_=ones[:, :])
            # ones columns of vaug/vaug8 hold 1/CA so denominators come out
            # pre-divided by CA (normalised at then carries a factor CA).
            nc.vector.memset(vaug[:, :, :, HD], 1.0 / CA)
            nc.vector.memset(vaug8[:, :, :, :, HD], 1.0 / CA)

            def ka():
                # HAM keep-alive: a 32-col dummy weight load (~27ns) resets
                # the PE idle timer so sparse phases don't get clock-gated.
                nc.tensor.ldweights(tri_sb[:, 0:32])

            def norm_unit(i, p, bpsum):
                """Broadcast 1/denominator and write normalised at/at8."""
                ssl = slice(i * SI, (i + 1) * SI)
                bc = bpsum.tile([128, SI], F32, tag="bc", name="bc")
                for l in range(2):
                    nc.tensor.matmul(
                        bc[64 * l:64 * l + 64, :],
                        ones_sb[64:65, 0:64],
                        rden[64:65, p, i, l, :],
                        start=True,
                        stop=True,
                    )
                ka()
                rbc = copool.tile([128, SI], F32, tag="rbc", name="rbc")
                nc.vector.reciprocal_approx_fast(rbc[:, :], bc[:, :])
                if i == 0:
                    nc.vector.tensor_mul(at[:, p, ssl], at[:, p, ssl], rbc[:, :])
                else:
                    nc.vector.tensor_mul(
                        at8[:, p // 2, p % 2, (i - 1) * SI:i * SI],
                        at[:, p, ssl],
                        rbc[:, :],
                    )

            def wo_chain(i, mt, cpsum):
                """Output projection for one 128-row block of out, stripe i."""
                ssl = slice(i * SI, (i + 1) * SI)
                ps = cpsum.tile([128, SI], F32, tag="cps", name="cps")
                if i == 0:
                    for kt in range(4):
                        nc.tensor.matmul(
                            ps[:, :],
                            wo_bf[:, kt, mt * 128:(mt + 1) * 128],
                            at[:, kt, ssl],
                            start=(kt == 0),
                            stop=(kt == 3),
                        )
                    oscale = 1.0 / CA
                else:
                    a8sl = slice((i - 1) * SI, i * SI)
                    for ktp in range(2):
                        nc.tensor.matmul(
                            ps[:, :],
                            wo8_sb[:, ktp, :, mt * 128:(mt + 1) * 128],
                            at8[:, ktp, :, a8sl],
                            start=(ktp == 0),
                            stop=(ktp == 1),
                            perf_mode=DR,
                        )
                    oscale = 1.0 / (CA * CO)
                osb = copool.tile([128, SI], F32, tag="osb", name="osb")
                nc.vector.tensor_scalar_mul(osb[:, :], ps[:, :], oscale)
                nc.sync.dma_start(out=out_v[:, mt, ssl], in_=osb[:, :])
                ka()

            with (
                tc.tile_pool(name="ptb", bufs=2) as ptbpool,
                tc.tile_pool(name="pt8", bufs=2) as pt8pool,
                tc.tile_pool(name="stps", bufs=2, space=MemorySpace.PSUM) as stps,
                tc.tile_pool(name="ops", bufs=2, space=MemorySpace.PSUM) as ops,
            ):

                def make_stripe(i, pair_done=None):
                    """Emission units for attention stripe i (all pairs).

                    Unit = one key-block-pair jp: 4 QK matmuls (row-tiled
                    concurrent head pairs), one big exp ACT, causal masks on
                    GpSimd for diagonal blocks, then PV (fp8 DoubleRow for
                    off-diagonal jp, bf16 for the diagonal ones).
                    """
                    si0 = i * SI
                    ssl = slice(si0, si0 + SI)
                    njs = 4 * i + 4
                    units = []
                    for p in range(NPAIR):
                        state = {}

                        def start_pair(p=p, state=state):
                            state["oa"] = ops.tile(
                                [65, SI], F32, tag="o2", name="oa"
                            )
                            state["ob"] = ops.tile(
                                [65, SI], F32, tag="o2", name="ob"
                            )

                        def unit2(j0, p=p, state=state, i=i, si0=si0):
                            """One emission unit = key blocks (j0, j0+1):
                            both QK pairs back-to-back (one 64-mode window),
                            both exps, masks, then the PV matmuls."""
                            njs = 4 * i + 4
                            nondiag = j0 < 4 * i
                            sts, pts, r0s = [], [], []
                            for j in (j0, j0 + 1):
                                d0 = j * SJ - si0
                                r0 = max(0, d0)
                                r0s.append(r0)
                                st = stps.tile(
                                    [128, 2, SI], F32, tag="st", name="st"
                                )
                                sts.append(st)
                                for l in range(2):
                                    lsl = slice(64 * l, 64 * l + 64)
                                    nc.tensor.matmul(
                                        st[:, l, r0:SI],
                                        kT[lsl, p, j * SJ:(j + 1) * SJ],
                                        qT[lsl, p, si0 + r0:si0 + SI],
                                        start=True,
                                        stop=True,
                                    )
                            if nondiag:
                                pt8 = pt8pool.tile(
                                    [128, 2, 2, SI], FP8, tag="pt8", name="pt8"
                                )
                                for jj in range(2):
                                    nc.scalar.activation(
                                        pt8[:, jj, :, :], sts[jj][:, :, :], AF.Exp
                                    )
                                for l in range(2):
                                    nc.tensor.matmul(
                                        (state["oa"] if l == 0 else state["ob"])[:, :],
                                        vaug8[:, j0 // 2, :, 2 * p + l, 0:HD + 1],
                                        pt8[:, :, l, :],
                                        start=(j0 == 0),
                                        stop=False,
                                        perf_mode=DR,
                                        skip_group_check=True,
                                    )
                            else:
                                for jj in range(2):
                                    j = j0 + jj
                                    r0 = r0s[jj]
                                    d0 = j * SJ - si0
                                    pt = ptbpool.tile(
                                        [128, 2, SI], BF16, tag="ptb", name="ptb"
                                    )
                                    pts.append(pt)
                                    nc.scalar.activation(
                                        pt[:, :, r0:SI], sts[jj][:, :, r0:SI], AF.Exp
                                    )
                                    if d0 >= 0:
                                        for l in range(2):
                                            nc.vector.tensor_mul(
                                                pt[:, l, d0:d0 + 128],
                                                pt[:, l, d0:d0 + 128],
                                                tri_sb[:, :],
                                            )
                                for jj in range(2):
                                    j = j0 + jj
                                    r0 = r0s[jj]
                                    for l in range(2):
                                        nc.tensor.matmul(
                                            (state["oa"] if l == 0 else state["ob"])[:, r0:SI],
                                            vaug[:, j, 2 * p + l, :],
                                            pts[jj][:, l, r0:SI],
                                            start=(j == 0),
                                            stop=(j == njs - 1),
                                            skip_group_check=True,
                                        )

                        def unit2_ka(j0, u=None, i=i):
                            u(j0)
                            npulse = 2 if i < 2 else 4
                            for _ in range(npulse):
                                ka()

                        def end_pair(p=p, state=state, ssl=ssl, i=i):
                            ka()
                            for l in range(2):
                                o2 = state["oa"] if l == 0 else state["ob"]
                                nc.vector.tensor_copy(
                                    at[64 * l:64 * l + 64, p, ssl], o2[0:HD, :]
                                )
                                nc.vector.tensor_copy(
                                    rden[64:65, p, i, l, :], o2[HD:HD + 1, :]
                                )
                            ka()

                        units.append(start_pair)
                        for j0 in range(0, njs, 2):
                            units.append(lambda j0=j0, u=unit2: unit2_ka(j0, u))
                        units.append(end_pair)
                        if pair_done is not None:
                            units.append(lambda p=p: pair_done(p))
                    return units

                def emit_interleaved(units, fillers):
                    """Emit units with fillers distributed evenly between."""
                    U, F = len(units), len(fillers)
                    fi = 0
                    for k, u in enumerate(units):
                        u()
                        want = (k + 1) * F // U
                        while fi < want:
                            fillers[fi]()
                            fi += 1
                    while fi < F:
                        fillers[fi]()
                        fi += 1

                with (
                    tc.tile_pool(name="ax", bufs=3) as xpool,
                    tc.tile_pool(name="ax0", bufs=2) as x0pool,
                    tc.tile_pool(name="aw", bufs=1) as wpool,
                    tc.tile_pool(name="aps", bufs=1, space=MemorySpace.PSUM) as apsum,
                ):
                    wq_sb = wpool.tile([128, KC, DG], BF16, tag="w0")
                    wk_sb = wpool.tile([128, KC, DG], BF16, tag="wk")
                    wv_sb = wpool.tile([128, KC, DG], BF16, tag="w0")
                    w8q_sb = wpool.tile([128, 4, 2, DG], FP8, tag="w8q")
                    w8k_sb = wpool.tile([128, 4, 2, DG], FP8, tag="w8k")
                    w8v_sb = wpool.tile([128, 4, 2, DG], FP8, tag="w8v")
                    w8_sb = {"q": w8q_sb, "k": w8k_sb, "v": w8v_sb}
                    x8_view = {"q": x8q_v, "k": x8k_v, "v": x8v_v}
                    x8_tiles = {}
                    pscale = {"q": 1.0 / CQ, "k": 1.0 / CK, "v": 1.0 / CV}

                    def dma_x8(t, n):
                        xt = xpool.tile(
                            [128, 4, 2, AC], FP8, tag="x8", name=f"x8_{t}{n}"
                        )
                        nc.sync.dma_start(
                            out=xt[:], in_=x8_view[t][:, :, :, n * AC:(n + 1) * AC]
                        )
                        x8_tiles[(t, n)] = xt

                    def chain_qk8(t, n, p):
                        """fp8 DoubleRow q/k projection chain, chunks 1-3."""
                        sl = slice(n * AC, (n + 1) * AC)
                        xt = x8_tiles[(t, n)]
                        ps = apsum.tile([128, AC], F32, tag="aps", name="aps")
                        for kcp in range(4):
                            nc.tensor.matmul(
                                ps[:, :],
                                w8_sb[t][:, kcp, :, p * 128:(p + 1) * 128],
                                xt[:, kcp, :, :],
                                start=(kcp == 0),
                                stop=(kcp == 3),
                                perf_mode=DR,
                            )
                        nc.vector.tensor_scalar_mul(
                            (qT if t == "q" else kT)[:, p, sl], ps[:, :], pscale[t]
                        )

                    def chain_v8(n, mm):
                        """fp8 DoubleRow v projection, one 128-seq block."""
                        xt = x8_tiles[("v", n)]
                        j = n * (AC // 128) + mm
                        ps = apsum.tile([128, DG], F32, tag="aps", name="apsv")
                        for kcp in range(4):
                            nc.tensor.matmul(
                                ps[:, :],
                                xt[:, kcp, :, mm * 128:(mm + 1) * 128],
                                w8_sb["v"][:, kcp, :, :],
                                start=(kcp == 0),
                                stop=(kcp == 3),
                                perf_mode=DR,
                            )
                        nc.vector.tensor_scalar_mul(
                            vaug[:, j, :, 0:HD], ps[:, :], pscale["v"]
                        )
                        if j < 12:
                            nc.vector.tensor_scalar_mul(
                                vaug8[:, j // 2, j % 2, :, 0:HD],
                                ps[:, :],
                                pscale["v"],
                            )

                    def chunk_fillers(n):
                        fs = [lambda t=t, n=n: dma_x8(t, n) for t in ("q", "k", "v")]
                        for p in range(NPAIR):
                            fs.append(lambda p=p, n=n: chain_qk8("q", n, p))
                        for p in range(NPAIR):
                            fs.append(lambda p=p, n=n: chain_qk8("k", n, p))
                        for mm in range(AC // 128):
                            fs.append(lambda mm=mm, n=n: chain_v8(n, mm))
                        return fs

                    # chunk 0: stream the q projection kc-major behind
                    # per-kc DMA slices so the first matmul fires after the
                    # first 128KB lands instead of the full 0.5MB.
                    xt0 = x0pool.tile([128, KC, AC], BF16, tag="x0", name="x_q0")
                    for kc in range(KC):
                        nc.sync.dma_start(
                            out=wq_sb[:, kc, :], in_=wq_v[:, kc, :]
                        )
                        nc.sync.dma_start(
                            out=xt0[:, kc, :], in_=xq0_v[:, kc, :]
                        )
                    # HAM warmup: ~4.5us of throwaway matmuls gated on the
                    # tiny tri/ones DMAs so they run while the first input
                    # slices land and hand over to the q0 chains with no gap.
                    warm = apsum.tile([64, 128], F32, tag="aps", name="warm")
                    for _ in range(44):
                        nc.tensor.matmul(
                            warm[:, :], ones_sb[:, :], tri_sb[:, :],
                            start=True, stop=True,
                        )
                    for ph in range(2):
                        pss = [
                            apsum.tile([128, AC], F32, tag="q0ps", name="q0ps")
                            for _ in range(2)
                        ]
                        for kc in range(KC):
                            for pi in range(2):
                                p = 2 * ph + pi
                                nc.tensor.matmul(
                                    pss[pi][:, :],
                                    wq_sb[:, kc, p * 128:(p + 1) * 128],
                                    xt0[:, kc, :],
                                    start=(kc == 0),
                                    stop=(kc == KC - 1),
                                )
                        for pi in range(2):
                            nc.vector.tensor_copy(
                                qT[:, 2 * ph + pi, 0:AC], pss[pi][:, :]
                            )
                    # chunk-0 k and v (bf16)
                    xtk0 = x0pool.tile([128, KC, AC], BF16, tag="x0", name="x_k0")
                    nc.sync.dma_start(out=xtk0[:], in_=xk0_v[:, :, :])
                    nc.sync.dma_start(out=wk_sb[:], in_=wk_v[:, :, :])
                    nc.sync.dma_start(out=wv_sb[:], in_=wv_v[:, :, :])
                    xtv0 = x0pool.tile([128, KC, AC], BF16, tag="x0", name="x_v0")
                    nc.sync.dma_start(out=xtv0[:], in_=xv0_v[:, :, :])
                    nc.sync.dma_start(out=w8q_sb[:], in_=w8q_v[:, :, :, :])
                    nc.sync.dma_start(out=w8k_sb[:], in_=w8k_v[:, :, :, :])
                    nc.sync.dma_start(out=w8v_sb[:], in_=w8v_v[:, :, :, :])

                    def chain_k0(p):
                        ps = apsum.tile([128, AC], F32, tag="aps", name="apsk0")
                        for kc in range(KC):
                            nc.tensor.matmul(
                                ps[:, :],
                                wk_sb[:, kc, p * 128:(p + 1) * 128],
                                xtk0[:, kc, :],
                                start=(kc == 0),
                                stop=(kc == KC - 1),
                            )
                        nc.vector.tensor_copy(kT[:, p, 0:AC], ps[:, :])

                    def chain_v0(mm):
                        ps = apsum.tile([128, DG], F32, tag="aps", name="apsv0")
                        for kc in range(KC):
                            nc.tensor.matmul(
                                ps[:, :],
                                xtv0[:, kc, mm * 128:(mm + 1) * 128],
                                wv_sb[:, kc, :],
                                start=(kc == 0),
                                stop=(kc == KC - 1),
                            )
                        nc.vector.tensor_copy(vaug[:, mm, :, 0:HD], ps[:, :])
                        nc.vector.tensor_copy(
                            vaug8[:, mm // 2, mm % 2, :, 0:HD], ps[:, :]
                        )

                    for p in range(NPAIR):
                        chain_k0(p)
                    for mm in range(AC // 128):
                        chain_v0(mm)

                    emit_interleaved(make_stripe(0), chunk_fillers(1))
                    emit_interleaved(make_stripe(1), chunk_fillers(2))
                    emit_interleaved(make_stripe(2), chunk_fillers(3))

                # ---- last stripe interleaves with normalise + Wo (1-buf
                # ---- psum pools; stalls absorb into exp waits)
                with (
                    tc.tile_pool(name="cps1", bufs=1, space=MemorySpace.PSUM) as cps1,
                    tc.tile_pool(name="bps1", bufs=1, space=MemorySpace.PSUM) as bps1,
                ):
                    c_fillers = [
                        lambda: nc.sync.dma_start(out=wo_bf[:], in_=wo_v[:, :, :]),
                        lambda: nc.sync.dma_start(out=wo8_sb[:], in_=wo8_v[:, :, :, :]),
                    ]
                    for i in range(3):
                        for p in range(NPAIR):
                            c_fillers.append(
                                lambda i=i, p=p: norm_unit(i, p, bps1)
                            )
                        for mt in range(8):
                            c_fillers.append(
                                lambda i=i, mt=mt: wo_chain(i, mt, cps1)
                            )

                    emit_interleaved(
                        make_stripe(3, pair_done=lambda p: norm_unit(3, p, bps1)),
                        c_fillers,
                    )

            # final output stripe with room to pipeline (st/o2 closed)
            with (
                tc.tile_pool(name="cps2", bufs=4, space=MemorySpace.PSUM) as cps2,
            ):
                for mt in range(8):
                    wo_chain(3, mt, cps2)

    nc.compile()
    return nc


_NC_CACHE = {}


def _get_nc():
    if "nc" not in _NC_CACHE:
        _NC_CACHE["nc"] = _build_core_kernel()
    return _NC_CACHE["nc"]


def make_in_maps(query, key, value, Wq, Wk, Wv, Wo):
    import ml_dtypes

    bf = ml_dtypes.bfloat16
    f8 = ml_dtypes.float8_e4m3
    tri = (np.arange(128)[None, :] >= np.arange(128)[:, None]).astype(bf)
    ones128 = np.ones((128, 64), bf)
    # per-batch transposed activations (shared by the batch's two cores):
    # chunk 0 in bf16, the full sequence in fp8
    xq0_b, xk0_b, xv0_b, x8q_b, x8k_b, x8v_b = [], [], [], [], [], []
    for b in range(B):
        for src, lo, hi in ((query, xq0_b, x8q_b), (key, xk0_b, x8k_b),
                            (value, xv0_b, x8v_b)):
            xt = np.ascontiguousarray(src[b].T)
            lo.append(xt[:, :AC].astype(bf))
            hi.append(xt.astype(f8))
    w_g = []
    for g in range(2):
        rows = slice(g * DG, (g + 1) * DG)
        wq_g = Wq[rows, :] * SCALE
        w_g.append({
            "wq": np.ascontiguousarray(wq_g.T).astype(bf),
            "wk": np.ascontiguousarray(Wk[rows, :].T).astype(bf),
            "wv": np.ascontiguousarray(Wv[rows, :].T).astype(bf),
            "w8q": np.ascontiguousarray((wq_g * CQ).T).astype(f8),
            "w8k": np.ascontiguousarray((Wk[rows, :] * CK).T).astype(f8),
            "w8v": np.ascontiguousarray((Wv[rows, :] * CV).T).astype(f8),
            "wo": np.ascontiguousarray(Wo[:, rows].T).astype(bf),
            "wo8": np.ascontiguousarray((Wo[:, rows] * CO).T).astype(f8),
        })
    in_maps = []
    for c in range(NCORES):
        b, g = c // 2, c % 2
        in_maps.append({
            "xq0": xq0_b[b], "xk0": xk0_b[b], "xv0": xv0_b[b],
            "x8q": x8q_b[b], "x8k": x8k_b[b], "x8v": x8v_b[b],
            **w_g[g], "tri": tri, "ones128": ones128,
        })
    return in_maps


def kernel(query, key, value, attn_mask, Wq, Wk, Wv, Wo):
    global LAST_RESULTS
    from concourse.bass_utils import run_bass_kernel_spmd

    query = np.asarray(query, np.float32)
    key = np.asarray(key, np.float32)
    value = np.asarray(value, np.float32)
    Wq = np.asarray(Wq, np.float32)
    Wk = np.asarray(Wk, np.float32)
    Wv = np.asarray(Wv, np.float32)
    Wo = np.asarray(Wo, np.float32)

    nc = _get_nc()
    in_maps = make_in_maps(query, key, value, Wq, Wk, Wv, Wo)
    res = run_bass_kernel_spmd(
        nc,
        in_maps,
        core_ids=list(range(NCORES)),
        trace=bool(int(os.environ.get("KERNEL_TRACE", "0"))),
    )
    LAST_RESULTS = res

    full = np.empty((B, S, DIM), np.float32)
    for b in range(B):
        full[b] = (res.results[2 * b]["out"] + res.results[2 * b + 1]["out"]).T
    return full


# revision 22
# speedup vs baseline: 1.0469x; 1.0193x over previous
"""Distributed causal attention kernel for 8 TRN2 NeuronCores.

Sharding: core c -> (batch b = c//2, head-group g = c%2).  Each core
computes attention for its batch over 8 of the 16 heads plus the partial
output projection (row-parallel Wo); the host sums the two partials per
batch and transposes back.

v3: position-hybrid fp8.  Numerics are bf16 for everything that touches
query positions 0-511 (whose outputs average over few keys and dominate
max-error) and fp8e4m3 + DoubleRow matmuls elsewhere:
  - q/k/v projections: seq chunk 0 bf16, chunks 1-3 fp8 (2 k-rows/cycle)
  - PV: diagonal key-blocks bf16 (also carries the causal mask),
    off-diagonal blocks fp8 DoubleRow packing 2 key-blocks per matmul
  - Wo: query stripe 0 bf16, stripes 1-3 fp8 DoubleRow
  - QK stays bf16: its cost is N-bound and the K=64 head pairs already
    run concurrently on the row-tiled (64x128) PE array.
Scales: wq carries SCALE*CQ, wk CK, wv CV, wo8 CO; vaug ones col = 1/CA
so denominators come out pre-divided and at picks up a factor CA.
Output evac rescales by 1/CA (stripe 0) or 1/(CA*CO) (stripes 1-3).

Exp is evaluated on ScalarE in one activation per (pair, block-pair):
st PSUM is [128, 2, 2, SI] (two key blocks x two heads) so each ACT
covers up to 2048 elements, amortising the ~352-cycle ACT overhead.
Causal masking of diagonal blocks runs on GpSimd (otherwise idle).
"""

import os

import numpy as np

import concourse.bass as bass
import concourse.tile as tile
from concourse import bacc, mybir
from concourse.bass import MemorySpace

F32 = mybir.dt.float32
BF16 = mybir.dt.bfloat16
FP8 = mybir.dt.float8e4
AF = mybir.ActivationFunctionType
DR = mybir.MatmulPerfMode.DoubleRow

B, S, DIM, H = 4, 2048, 1024, 16
HD = DIM // H          # 64
SCALE = HD ** -0.5
NCORES = 8
DG = DIM // 2          # 512 head dims per core (8 heads)
NPAIR = 4              # head pairs per core
SI = 512               # si chunk (query positions per attention stripe)
SJ = 128               # sj chunk (key positions per matmul)
NSI = S // SI          # 4
AC = 512               # seq chunk for projections
NAC = S // AC          # 4
KC = DIM // 128        # 8 contraction chunks for bf16 projections
CA = 8.0               # at scale (ones col = 1/CA)
CQ = 512.0             # fp8 weight scales
CK = 64.0
CV = 64.0
CO = 64.0

LAST_RESULTS = None


def _build_core_kernel():
    nc = bacc.Bacc(
        "TRN2", target_bir_lowering=False, debug=False, num_devices=NCORES
    )

    # chunk-0 (positions 0-511) activations, bf16
    xq0 = nc.dram_tensor("xq0", [DIM, AC], BF16, kind="ExternalInput").ap()
    xk0 = nc.dram_tensor("xk0", [DIM, AC], BF16, kind="ExternalInput").ap()
    xv0 = nc.dram_tensor("xv0", [DIM, AC], BF16, kind="ExternalInput").ap()
    # full activations fp8 (only chunks 1-3 are read)
    x8q = nc.dram_tensor("x8q", [DIM, S], FP8, kind="ExternalInput").ap()
    x8k = nc.dram_tensor("x8k", [DIM, S], FP8, kind="ExternalInput").ap()
    x8v = nc.dram_tensor("x8v", [DIM, S], FP8, kind="ExternalInput").ap()
    wq = nc.dram_tensor("wq", [DIM, DG], BF16, kind="ExternalInput").ap()
    wk = nc.dram_tensor("wk", [DIM, DG], BF16, kind="ExternalInput").ap()
    wv = nc.dram_tensor("wv", [DIM, DG], BF16, kind="ExternalInput").ap()
    w8q = nc.dram_tensor("w8q", [DIM, DG], FP8, kind="ExternalInput").ap()
    w8k = nc.dram_tensor("w8k", [DIM, DG], FP8, kind="ExternalInput").ap()
    w8v = nc.dram_tensor("w8v", [DIM, DG], FP8, kind="ExternalInput").ap()
    wo = nc.dram_tensor("wo", [DG, DIM], BF16, kind="ExternalInput").ap()
    wo8 = nc.dram_tensor("wo8", [DG, DIM], FP8, kind="ExternalInput").ap()
    tri = nc.dram_tensor("tri", [128, 128], BF16, kind="ExternalInput").ap()
    ones = nc.dram_tensor("ones128", [128, 64], BF16, kind="ExternalInput").ap()
    out = nc.dram_tensor("out", [DIM, S], F32, kind="ExternalOutput").ap()

    # partition-tiled DRAM views
    xq0_v = xq0.rearrange("(kc p) s -> p kc s", p=128)   # [128, 8, 512]
    xk0_v = xk0.rearrange("(kc p) s -> p kc s", p=128)
    xv0_v = xv0.rearrange("(kc p) s -> p kc s", p=128)
    x8q_v = x8q.rearrange("(kcp two p) s -> p kcp two s", p=128, two=2)
    x8k_v = x8k.rearrange("(kcp two p) s -> p kcp two s", p=128, two=2)
    x8v_v = x8v.rearrange("(kcp two p) s -> p kcp two s", p=128, two=2)
    wq_v = wq.rearrange("(kc p) m -> p kc m", p=128)     # [128, 8, 512]
    wk_v = wk.rearrange("(kc p) m -> p kc m", p=128)
    wv_v = wv.rearrange("(kc p) m -> p kc m", p=128)
    w8q_v = w8q.rearrange("(kcp two p) m -> p kcp two m", p=128, two=2)
    w8k_v = w8k.rearrange("(kcp two p) m -> p kcp two m", p=128, two=2)
    w8v_v = w8v.rearrange("(kcp two p) m -> p kcp two m", p=128, two=2)
    wo_v = wo.rearrange("(kt p) m -> p kt m", p=128)     # [128, 4, 1024]
    wo8_v = wo8.rearrange("(ktp two p) m -> p ktp two m", p=128, two=2)
    out_v = out.rearrange("(mt p) s -> p mt s", p=128)   # [128, 8, 2048]

    with tile.TileContext(nc) as tc:
        with (
            tc.tile_pool(name="persist", bufs=1) as persist,
            tc.tile_pool(name="cw", bufs=1) as cwpool,
            tc.tile_pool(name="co", bufs=2) as copool,
        ):
            # persistent SBUF tensors
            qT = persist.tile([128, NPAIR, S], BF16)        # [64l+d, pair, si]
            kT = persist.tile([128, NPAIR, S], BF16)
            vaug = persist.tile([128, S // SJ, 8, HD + 1], BF16)  # [sj, j, h, d|1/CA]
            vaug8 = persist.tile([128, 6, 2, 8, HD + 2], FP8)     # [sj, jp, jj, h, d|1/CA|pad]
            at = persist.tile([128, NPAIR, S], BF16)        # unnorm A.T * CA-pre
            at8 = persist.tile([128, 2, 2, 3 * SI], FP8)    # normed, stripes 1-3
            rden = persist.tile([65, NPAIR, NSI, 2, SI], BF16)  # raw denom/CA @ p64
            tri_sb = persist.tile([128, 128], BF16)
            ones_sb = persist.tile([128, 64], BF16)
            warm_sb = persist.tile([128, 128], BF16)
            wo_bf = cwpool.tile([128, 4, DIM], BF16, tag="wo16")
            wo8_sb = cwpool.tile([128, 2, 2, DIM], FP8, tag="wo8")

            nc.vector.memset(warm_sb[:], 0.125)
            nc.sync.dma_start(out=tri_sb[:], in_=tri[:, :])
            nc.sync.dma_start(out=ones_sb[:], in_=ones[:, :])
            # ones columns of vaug/vaug8 hold 1/CA so denominators come out
            # pre-divided by CA (normalised at then carries a factor CA).
            nc.vector.memset(vaug[:, :, :, HD], 1.0 / CA)
            nc.vector.memset(vaug8[:, :, :, :, HD], 1.0 / CA)

            def ka():
                # HAM keep-alive: a 32-col dummy weight load (~27ns) resets
                # the PE idle timer so sparse phases don't get clock-gated.
                nc.tensor.ldweights(tri_sb[:, 0:32])

            def norm_unit(i, p, bpsum):
                """Broadcast 1/denominator and write normalised at/at8."""
                ssl = slice(i * SI, (i + 1) * SI)
                bc = bpsum.tile([128, SI], F32, tag="bc", name="bc")
                for l in range(2):
                    nc.tensor.matmul(
                        bc[64 * l:64 * l + 64, :],
                        ones_sb[64:65, 0:64],
                        rden[64:65, p, i, l, :],
                        start=True,
                        stop=True,
                    )
                ka()
                rbc = copool.tile([128, SI], F32, tag="rbc", name="rbc")
                nc.vector.reciprocal_approx_fast(rbc[:, :], bc[:, :])
                if i == 0:
                    nc.vector.tensor_mul(at[:, p, ssl], at[:, p, ssl], rbc[:, :])
                else:
                    nc.vector.tensor_mul(
                        at8[:, p // 2, p % 2, (i - 1) * SI:i * SI],
                        at[:, p, ssl],
                        rbc[:, :],
                    )

            def wo_chain(i, mt, cpsum):
                """Output projection for one 128-row block of out, stripe i."""
                ssl = slice(i * SI, (i + 1) * SI)
                ps = cpsum.tile([128, SI], F32, tag="cps", name="cps")
                if i == 0:
                    for kt in range(4):
                        nc.tensor.matmul(
                            ps[:, :],
                            wo_bf[:, kt, mt * 128:(mt + 1) * 128],
                            at[:, kt, ssl],
                            start=(kt == 0),
                            stop=(kt == 3),
                        )
                    oscale = 1.0 / CA
                else:
                    a8sl = slice((i - 1) * SI, i * SI)
                    for ktp in range(2):
                        nc.tensor.matmul(
                            ps[:, :],
                            wo8_sb[:, ktp, :, mt * 128:(mt + 1) * 128],
                            at8[:, ktp, :, a8sl],
                            start=(ktp == 0),
                            stop=(ktp == 1),
                            perf_mode=DR,
                        )
                    oscale = 1.0 / (CA * CO)
                osb = copool.tile([128, SI], F32, tag="osb", name="osb")
                nc.vector.tensor_scalar_mul(osb[:, :], ps[:, :], oscale)
                nc.sync.dma_start(out=out_v[:, mt, ssl], in_=osb[:, :])
                ka()

            with (
                tc.tile_pool(name="ptb", bufs=3) as ptbpool,
                tc.tile_pool(name="pt8", bufs=3) as pt8pool,
                tc.tile_pool(name="stps", bufs=2, space=MemorySpace.PSUM) as stps,
                tc.tile_pool(name="ops", bufs=2, space=MemorySpace.PSUM) as ops,
            ):

                def make_stripe(i, pair_done=None):
                    """Emission units for attention stripe i (all pairs).

                    Unit = one key-block-pair jp: 4 QK matmuls (row-tiled
                    concurrent head pairs), one big exp ACT, causal masks on
                    GpSimd for diagonal blocks, then PV (fp8 DoubleRow for
                    off-diagonal jp, bf16 for the diagonal ones).
                    """
                    si0 = i * SI
                    ssl = slice(si0, si0 + SI)
                    njs = 4 * i + 4
                    units = []
                    for p in range(NPAIR):
                        state = {}

                        def start_pair(p=p, state=state):
                            state["oa"] = ops.tile(
                                [65, SI], F32, tag="o2", name="oa"
                            )
                            state["ob"] = ops.tile(
                                [65, SI], F32, tag="o2", name="ob"
                            )

                        def unit2(j0, p=p, state=state, i=i, si0=si0):
                            """One emission unit = key blocks (j0, j0+1):
                            both QK pairs back-to-back (one 64-mode window),
                            both exps, masks, then the PV matmuls."""
                            njs = 4 * i + 4
                            nondiag = j0 < 4 * i
                            sts, pts, r0s = [], [], []
                            for j in (j0, j0 + 1):
                                d0 = j * SJ - si0
                                r0 = max(0, d0)
                                r0s.append(r0)
                                st = stps.tile(
                                    [128, 2, SI], F32, tag="st", name="st"
                                )
                                sts.append(st)
                                for l in range(2):
                                    lsl = slice(64 * l, 64 * l + 64)
                                    nc.tensor.matmul(
                                        st[:, l, r0:SI],
                                        kT[lsl, p, j * SJ:(j + 1) * SJ],
                                        qT[lsl, p, si0 + r0:si0 + SI],
                                        start=True,
                                        stop=True,
                                    )
                            if nondiag:
                                pt8 = pt8pool.tile(
                                    [128, 2, 2, SI], FP8, tag="pt8", name="pt8"
                                )
                                for jj in range(2):
                                    nc.scalar.activation(
                                        pt8[:, jj, :, :], sts[jj][:, :, :], AF.Exp
                                    )
                                for l in range(2):
                                    nc.tensor.matmul(
                                        (state["oa"] if l == 0 else state["ob"])[:, :],
                                        vaug8[:, j0 // 2, :, 2 * p + l, 0:HD + 1],
                                        pt8[:, :, l, :],
                                        start=(j0 == 0),
                                        stop=False,
                                        perf_mode=DR,
                                        skip_group_check=True,
                                    )
                            else:
                                for jj in range(2):
                                    j = j0 + jj
                                    r0 = r0s[jj]
                                    d0 = j * SJ - si0
                                    pt = ptbpool.tile(
                                        [128, 2, SI], BF16, tag="ptb", name="ptb"
                                    )
                                    pts.append(pt)
                                    nc.scalar.activation(
                                        pt[:, :, r0:SI], sts[jj][:, :, r0:SI], AF.Exp
                                    )
                                    if d0 >= 0:
                                        for l in range(2):
                                            nc.vector.tensor_mul(
                                                pt[:, l, d0:d0 + 128],
                                                pt[:, l, d0:d0 + 128],
                                                tri_sb[:, :],
                                            )
                                for jj in range(2):
                                    j = j0 + jj
                                    r0 = r0s[jj]
                                    for l in range(2):
                                        nc.tensor.matmul(
                                            (state["oa"] if l == 0 else state["ob"])[:, r0:SI],
                                            vaug[:, j, 2 * p + l, :],
                                            pts[jj][:, l, r0:SI],
                                            start=(j == 0),
                                            stop=(j == njs - 1),
                                            skip_group_check=True,
                                        )

                        def unit2_ka(j0, u=None, i=i):
                            u(j0)
                            npulse = 2 if i < 2 else 4
                            for _ in range(npulse):
                                ka()

                        def end_pair(p=p, state=state, ssl=ssl, i=i):
                            ka()
                            for l in range(2):
                                o2 = state["oa"] if l == 0 else state["ob"]
                                nc.vector.tensor_copy(
                                    at[64 * l:64 * l + 64, p, ssl], o2[0:HD, :]
                                )
                                nc.vector.tensor_copy(
                                    rden[64:65, p, i, l, :], o2[HD:HD + 1, :]
                                )
                            ka()

                        units.append(start_pair)
                        for j0 in range(0, njs, 2):
                            units.append(lambda j0=j0, u=unit2: unit2_ka(j0, u))
                        units.append(end_pair)
                        if pair_done is not None:
                            units.append(lambda p=p: pair_done(p))
                    return units

                def emit_interleaved(units, fillers):
                    """Emit units with fillers distributed evenly between."""
                    U, F = len(units), len(fillers)
                    fi = 0
                    for k, u in enumerate(units):
                        u()
                        want = (k + 1) * F // U
                        while fi < want:
                            fillers[fi]()
                            fi += 1
                    while fi < F:
                        fillers[fi]()
                        fi += 1

                with (
                    tc.tile_pool(name="ax", bufs=3) as xpool,
                    tc.tile_pool(name="ax0", bufs=2) as x0pool,
                    tc.tile_pool(name="aw", bufs=1) as wpool,
                    tc.tile_pool(name="aps", bufs=1, space=MemorySpace.PSUM) as apsum,
                ):
                    wq_sb = wpool.tile([128, KC, DG], BF16, tag="w0")
                    wk_sb = wpool.tile([128, KC, DG], BF16, tag="wk")
                    wv_sb = wpool.tile([128, KC, DG], BF16, tag="w0")
                    w8q_sb = wpool.tile([128, 4, 2, DG], FP8, tag="w8q")
                    w8k_sb = wpool.tile([128, 4, 2, DG], FP8, tag="w8k")
                    w8v_sb = wpool.tile([128, 4, 2, DG], FP8, tag="w8v")
                    w8_sb = {"q": w8q_sb, "k": w8k_sb, "v": w8v_sb}
                    x8_view = {"q": x8q_v, "k": x8k_v, "v": x8v_v}
                    x8_tiles = {}
                    pscale = {"q": 1.0 / CQ, "k": 1.0 / CK, "v": 1.0 / CV}

                    def dma_x8(t, n):
                        xt = xpool.tile(
                            [128, 4, 2, AC], FP8, tag="x8", name=f"x8_{t}{n}"
                        )
                        nc.sync.dma_start(
                            out=xt[:], in_=x8_view[t][:, :, :, n * AC:(n + 1) * AC]
                        )
                        x8_tiles[(t, n)] = xt

                    def chain_qk8(t, n, p):
                        """fp8 DoubleRow q/k projection chain, chunks 1-3."""
                        sl = slice(n * AC, (n + 1) * AC)
                        xt = x8_tiles[(t, n)]
                        ps = apsum.tile([128, AC], F32, tag="aps", name="aps")
                        for kcp in range(4):
                            nc.tensor.matmul(
                                ps[:, :],
                                w8_sb[t][:, kcp, :, p * 128:(p + 1) * 128],
                                xt[:, kcp, :, :],
                                start=(kcp == 0),
                                stop=(kcp == 3),
                                perf_mode=DR,
                            )
                        nc.vector.tensor_scalar_mul(
                            (qT if t == "q" else kT)[:, p, sl], ps[:, :], pscale[t]
                        )

                    def chain_v8(n, mm):
                        """fp8 DoubleRow v projection, one 128-seq block."""
                        xt = x8_tiles[("v", n)]
                        j = n * (AC // 128) + mm
                        ps = apsum.tile([128, DG], F32, tag="aps", name="apsv")
                        for kcp in range(4):
                            nc.tensor.matmul(
                                ps[:, :],
                                xt[:, kcp, :, mm * 128:(mm + 1) * 128],
                                w8_sb["v"][:, kcp, :, :],
                                start=(kcp == 0),
                                stop=(kcp == 3),
                                perf_mode=DR,
                            )
                        nc.vector.tensor_scalar_mul(
                            vaug[:, j, :, 0:HD], ps[:, :], pscale["v"]
                        )
                        if j < 12:
                            nc.vector.tensor_scalar_mul(
                                vaug8[:, j // 2, j % 2, :, 0:HD],
                                ps[:, :],
                                pscale["v"],
                            )

                    def chunk_fillers(n):
                        fs = [lambda t=t, n=n: dma_x8(t, n) for t in ("q", "k", "v")]
                        for p in range(NPAIR):
                            fs.append(lambda p=p, n=n: chain_qk8("q", n, p))
                        for p in range(NPAIR):
                            fs.append(lambda p=p, n=n: chain_qk8("k", n, p))
                        for mm in range(AC // 128):
                            fs.append(lambda mm=mm, n=n: chain_v8(n, mm))
                        return fs

                    # chunk 0: stream the q projection kc-major behind
                    # per-kc DMA slices so the first matmul fires after the
                    # first 128KB lands instead of the full 0.5MB.
                    xt0 = x0pool.tile([128, KC, AC], BF16, tag="x0", name="x_q0")
                    for kc in range(KC):
                        nc.sync.dma_start(
                            out=wq_sb[:, kc, :], in_=wq_v[:, kc, :]
                        )
                        nc.sync.dma_start(
                            out=xt0[:, kc, :], in_=xq0_v[:, kc, :]
                        )
                    # HAM warmup: ~4.5us of throwaway matmuls gated on the
                    # tiny tri/ones DMAs so they run while the first input
                    # slices land and hand over to the q0 chains with no gap.
                    warm = apsum.tile([64, 128], F32, tag="aps", name="warm")
                    for _ in range(44):
                        nc.tensor.matmul(
                            warm[:, :], ones_sb[:, :], tri_sb[:, :],
                            start=True, stop=True,
                        )
                    for ph in range(2):
                        pss = [
                            apsum.tile([128, AC], F32, tag="q0ps", name="q0ps")
                            for _ in range(2)
                        ]
                        for kc in range(KC):
                            for pi in range(2):
                                p = 2 * ph + pi
                                nc.tensor.matmul(
                                    pss[pi][:, :],
                                    wq_sb[:, kc, p * 128:(p + 1) * 128],
                                    xt0[:, kc, :],
                                    start=(kc == 0),
                                    stop=(kc == KC - 1),
                                )
                        for pi in range(2):
                            nc.vector.tensor_copy(
                                qT[:, 2 * ph + pi, 0:AC], pss[pi][:, :]
                            )
                    # chunk-0 k and v (bf16)
                    xtk0 = x0pool.tile([128, KC, AC], BF16, tag="x0", name="x_k0")
                    nc.sync.dma_start(out=xtk0[:], in_=xk0_v[:, :, :])
                    nc.sync.dma_start(out=wk_sb[:], in_=wk_v[:, :, :])
                    nc.sync.dma_start(out=wv_sb[:], in_=wv_v[:, :, :])
                    xtv0 = x0pool.tile([128, KC, AC], BF16, tag="x0", name="x_v0")
                    nc.sync.dma_start(out=xtv0[:], in_=xv0_v[:, :, :])
                    nc.sync.dma_start(out=w8q_sb[:], in_=w8q_v[:, :, :, :])
                    nc.sync.dma_start(out=w8k_sb[:], in_=w8k_v[:, :, :, :])
                    nc.sync.dma_start(out=w8v_sb[:], in_=w8v_v[:, :, :, :])

                    def chain_k0(p):
                        ps = apsum.tile([128, AC], F32, tag="aps", name="apsk0")
                        for kc in range(KC):
                            nc.tensor.matmul(
                                ps[:, :],
                                wk_sb[:, kc, p * 128:(p + 1) * 128],
                                xtk0[:, kc, :],
                                start=(kc == 0),
                                stop=(kc == KC - 1),
                            )
                        nc.vector.tensor_copy(kT[:, p, 0:AC], ps[:, :])

                    def chain_v0(mm):
                        ps = apsum.tile([128, DG], F32, tag="aps", name="apsv0")
                        for kc in range(KC):
                            nc.tensor.matmul(
                                ps[:, :],
                                xtv0[:, kc, mm * 128:(mm + 1) * 128],
                                wv_sb[:, kc, :],
                                start=(kc == 0),
                                stop=(kc == KC - 1),
                            )
                        nc.vector.tensor_copy(vaug[:, mm, :, 0:HD], ps[:, :])
                        nc.vector.tensor_copy(
                            vaug8[:, mm // 2, mm % 2, :, 0:HD], ps[:, :]
                        )

                    for p in range(NPAIR):
                        chain_k0(p)
                    for mm in range(AC // 128):
                        chain_v0(mm)

                    emit_interleaved(make_stripe(0), chunk_fillers(1))
                    emit_interleaved(make_stripe(1), chunk_fillers(2))
                    emit_interleaved(make_stripe(2), chunk_fillers(3))

                # ---- last stripe interleaves with normalise + Wo (1-buf
                # ---- psum pools; stalls absorb into exp waits)
                with (
                    tc.tile_pool(name="cps1", bufs=1, space=MemorySpace.PSUM) as cps1,
                    tc.tile_pool(name="bps1", bufs=1, space=MemorySpace.PSUM) as bps1,
                ):
                    c_fillers = [
                        lambda: nc.sync.dma_start(out=wo_bf[:], in_=wo_v[:, :, :]),
                        lambda: nc.sync.dma_start(out=wo8_sb[:], in_=wo8_v[:, :, :, :]),
                    ]
                    for i in range(3):
                        for p in range(NPAIR):
                            c_fillers.append(
                                lambda i=i, p=p: norm_unit(i, p, bps1)
                            )
                        for mt in range(8):
                            c_fillers.append(
                                lambda i=i, mt=mt: wo_chain(i, mt, cps1)
                            )

                    emit_interleaved(
                        make_stripe(3, pair_done=lambda p: norm_unit(3, p, bps1)),
                        c_fillers,
                    )

            # final output stripe with room to pipeline (st/o2 closed)
            with (
                tc.tile_pool(name="cps2", bufs=4, space=MemorySpace.PSUM) as cps2,
            ):
                for mt in range(8):
                    wo_chain(3, mt, cps2)

    nc.compile()
    return nc


_NC_CACHE = {}


def _get_nc():
    if "nc" not in _NC_CACHE:
        _NC_CACHE["nc"] = _build_core_kernel()
    return _NC_CACHE["nc"]


def make_in_maps(query, key, value, Wq, Wk, Wv, Wo):
    import ml_dtypes

    bf = ml_dtypes.bfloat16
    f8 = ml_dtypes.float8_e4m3
    tri = (np.arange(128)[None, :] >= np.arange(128)[:, None]).astype(bf)
    ones128 = np.ones((128, 64), bf)
    # per-batch transposed activations (shared by the batch's two cores):
    # chunk 0 in bf16, the full sequence in fp8
    xq0_b, xk0_b, xv0_b, x8q_b, x8k_b, x8v_b = [], [], [], [], [], []
    for b in range(B):
        for src, lo, hi in ((query, xq0_b, x8q_b), (key, xk0_b, x8k_b),
                            (value, xv0_b, x8v_b)):
            xt = np.ascontiguousarray(src[b].T)
            lo.append(xt[:, :AC].astype(bf))
            hi.append(xt.astype(f8))
    w_g = []
    for g in range(2):
        rows = slice(g * DG, (g + 1) * DG)
        wq_g = Wq[rows, :] * SCALE
        w_g.append({
            "wq": np.ascontiguousarray(wq_g.T).astype(bf),
            "wk": np.ascontiguousarray(Wk[rows, :].T).astype(bf),
            "wv": np.ascontiguousarray(Wv[rows, :].T).astype(bf),
            "w8q": np.ascontiguousarray((wq_g * CQ).T).astype(f8),
            "w8k": np.ascontiguousarray((Wk[rows, :] * CK).T).astype(f8),
            "w8v": np.ascontiguousarray((Wv[rows, :] * CV).T).astype(f8),
            "wo": np.ascontiguousarray(Wo[:, rows].T).astype(bf),
            "wo8": np.ascontiguousarray((Wo[:, rows] * CO).T).astype(f8),
        })
    in_maps = []
    for c in range(NCORES):
        b, g = c // 2, c % 2
        in_maps.append({
            "xq0": xq0_b[b], "xk0": xk0_b[b], "xv0": xv0_b[b],
            "x8q": x8q_b[b], "x8k": x8k_b[b], "x8v": x8v_b[b],
            **w_g[g], "tri": tri, "ones128": ones128,
        })
    return in_maps


def kernel(query, key, value, attn_mask, Wq, Wk, Wv, Wo):
    global LAST_RESULTS
    from concourse.bass_utils import run_bass_kernel_spmd

    query = np.asarray(query, np.float32)
    key = np.asarray(key, np.float32)
    value = np.asarray(value, np.float32)
    Wq = np.asarray(Wq, np.float32)
    Wk = np.asarray(Wk, np.float32)
    Wv = np.asarray(Wv, np.float32)
    Wo = np.asarray(Wo, np.float32)

    nc = _get_nc()
    in_maps = make_in_maps(query, key, value, Wq, Wk, Wv, Wo)
    res = run_bass_kernel_spmd(
        nc,
        in_maps,
        core_ids=list(range(NCORES)),
        trace=bool(int(os.environ.get("KERNEL_TRACE", "0"))),
    )
    LAST_RESULTS = res

    full = np.empty((B, S, DIM), np.float32)
    for b in range(B):
        full[b] = (res.results[2 * b]["out"] + res.results[2 * b + 1]["out"]).T
    return full


# revision 23
# speedup vs baseline: 1.0599x; 1.0123x over previous
"""Distributed causal attention kernel for 8 TRN2 NeuronCores.

Sharding: core c -> (batch b = c//2, head-group g = c%2).  Each core
computes attention for its batch over 8 of the 16 heads plus the partial
output projection (row-parallel Wo); the host sums the two partials per
batch and transposes back.

v3: position-hybrid fp8.  Numerics are bf16 for everything that touches
query positions 0-511 (whose outputs average over few keys and dominate
max-error) and fp8e4m3 + DoubleRow matmuls elsewhere:
  - q/k/v projections: seq chunk 0 bf16, chunks 1-3 fp8 (2 k-rows/cycle)
  - PV: diagonal key-blocks bf16 (also carries the causal mask),
    off-diagonal blocks fp8 DoubleRow packing 2 key-blocks per matmul
  - Wo: query stripe 0 bf16, stripes 1-3 fp8 DoubleRow
  - QK stays bf16: its cost is N-bound and the K=64 head pairs already
    run concurrently on the row-tiled (64x128) PE array.
Scales: wq carries SCALE*CQ, wk CK, wv CV, wo8 CO; vaug ones col = 1/CA
so denominators come out pre-divided and at picks up a factor CA.
Output evac rescales by 1/CA (stripe 0) or 1/(CA*CO) (stripes 1-3).

Exp is evaluated on ScalarE in one activation per (pair, block-pair):
st PSUM is [128, 2, 2, SI] (two key blocks x two heads) so each ACT
covers up to 2048 elements, amortising the ~352-cycle ACT overhead.
Causal masking of diagonal blocks runs on GpSimd (otherwise idle).
"""

import os

import numpy as np

import concourse.bass as bass
import concourse.tile as tile
from concourse import bacc, mybir
from concourse.bass import MemorySpace

F32 = mybir.dt.float32
BF16 = mybir.dt.bfloat16
FP8 = mybir.dt.float8e4
AF = mybir.ActivationFunctionType
DR = mybir.MatmulPerfMode.DoubleRow

B, S, DIM, H = 4, 2048, 1024, 16
HD = DIM // H          # 64
SCALE = HD ** -0.5
NCORES = 8
DG = DIM // 2          # 512 head dims per core (8 heads)
NPAIR = 4              # head pairs per core
SI = 512               # si chunk (query positions per attention stripe)
SJ = 128               # sj chunk (key positions per matmul)
NSI = S // SI          # 4
AC = 512               # seq chunk for projections
NAC = S // AC          # 4
KC = DIM // 128        # 8 contraction chunks for bf16 projections
CA = 8.0               # at scale (ones col = 1/CA)
CQ = 512.0             # fp8 weight scales
CK = 64.0
CV = 64.0
CO = 64.0

LAST_RESULTS = None


def _build_core_kernel():
    nc = bacc.Bacc(
        "TRN2", target_bir_lowering=False, debug=False, num_devices=NCORES
    )

    # chunk-0 (positions 0-511) activations, bf16
    xq0 = nc.dram_tensor("xq0", [DIM, AC], BF16, kind="ExternalInput").ap()
    xk0 = nc.dram_tensor("xk0", [DIM, AC], BF16, kind="ExternalInput").ap()
    xv0 = nc.dram_tensor("xv0", [DIM, AC], BF16, kind="ExternalInput").ap()
    # full activations fp8 (only chunks 1-3 are read)
    x8q = nc.dram_tensor("x8q", [DIM, S], FP8, kind="ExternalInput").ap()
    x8k = nc.dram_tensor("x8k", [DIM, S], FP8, kind="ExternalInput").ap()
    x8v = nc.dram_tensor("x8v", [DIM, S], FP8, kind="ExternalInput").ap()
    wq = nc.dram_tensor("wq", [DIM, DG], BF16, kind="ExternalInput").ap()
    wk = nc.dram_tensor("wk", [DIM, DG], BF16, kind="ExternalInput").ap()
    wv = nc.dram_tensor("wv", [DIM, DG], BF16, kind="ExternalInput").ap()
    w8q = nc.dram_tensor("w8q", [DIM, DG], FP8, kind="ExternalInput").ap()
    w8k = nc.dram_tensor("w8k", [DIM, DG], FP8, kind="ExternalInput").ap()
    w8v = nc.dram_tensor("w8v", [DIM, DG], FP8, kind="ExternalInput").ap()
    wo = nc.dram_tensor("wo", [DG, DIM], BF16, kind="ExternalInput").ap()
    wo8 = nc.dram_tensor("wo8", [DG, DIM], FP8, kind="ExternalInput").ap()
    tri = nc.dram_tensor("tri", [128, 128], BF16, kind="ExternalInput").ap()
    ones = nc.dram_tensor("ones128", [128, 64], BF16, kind="ExternalInput").ap()
    out = nc.dram_tensor("out", [DIM, S], F32, kind="ExternalOutput").ap()

    # partition-tiled DRAM views
    xq0_v = xq0.rearrange("(kc p) s -> p kc s", p=128)   # [128, 8, 512]
    xk0_v = xk0.rearrange("(kc p) s -> p kc s", p=128)
    xv0_v = xv0.rearrange("(kc p) s -> p kc s", p=128)
    x8q_v = x8q.rearrange("(kcp two p) s -> p kcp two s", p=128, two=2)
    x8k_v = x8k.rearrange("(kcp two p) s -> p kcp two s", p=128, two=2)
    x8v_v = x8v.rearrange("(kcp two p) s -> p kcp two s", p=128, two=2)
    wq_v = wq.rearrange("(kc p) m -> p kc m", p=128)     # [128, 8, 512]
    wk_v = wk.rearrange("(kc p) m -> p kc m", p=128)
    wv_v = wv.rearrange("(kc p) m -> p kc m", p=128)
    w8q_v = w8q.rearrange("(kcp two p) m -> p kcp two m", p=128, two=2)
    w8k_v = w8k.rearrange("(kcp two p) m -> p kcp two m", p=128, two=2)
    w8v_v = w8v.rearrange("(kcp two p) m -> p kcp two m", p=128, two=2)
    wo_v = wo.rearrange("(kt p) m -> p kt m", p=128)     # [128, 4, 1024]
    wo8_v = wo8.rearrange("(ktp two p) m -> p ktp two m", p=128, two=2)
    out_v = out.rearrange("(mt p) s -> p mt s", p=128)   # [128, 8, 2048]

    with tile.TileContext(nc) as tc:
        with (
            tc.tile_pool(name="persist", bufs=1) as persist,
            tc.tile_pool(name="cw", bufs=1) as cwpool,
            tc.tile_pool(name="co", bufs=2) as copool,
        ):
            # persistent SBUF tensors
            qT = persist.tile([128, NPAIR, S], BF16)        # [64l+d, pair, si]
            kT = persist.tile([128, NPAIR, S], BF16)
            vaug = persist.tile([128, S // SJ, 8, HD + 1], BF16)  # [sj, j, h, d|1/CA]
            vaug8 = persist.tile([128, 6, 2, 8, HD + 2], FP8)     # [sj, jp, jj, h, d|1/CA|pad]
            at = persist.tile([128, NPAIR, S], BF16)        # unnorm A.T * CA-pre
            at8 = persist.tile([128, 2, 2, 3 * SI], FP8)    # normed, stripes 1-3
            rden = persist.tile([65, NPAIR, NSI, 2, SI], BF16)  # raw denom/CA @ p64
            tri_sb = persist.tile([128, 128], BF16)
            ones_sb = persist.tile([128, 64], BF16)
            warm_sb = persist.tile([128, 128], BF16)
            wo_bf = cwpool.tile([128, 4, DIM], BF16, tag="wo16")
            wo8_sb = cwpool.tile([128, 2, 2, DIM], FP8, tag="wo8")

            nc.vector.memset(warm_sb[:], 0.125)
            nc.sync.dma_start(out=tri_sb[:], in_=tri[:, :])
            nc.sync.dma_start(out=ones_sb[:], in_=ones[:, :])
            # ones columns of vaug/vaug8 hold 1/CA so denominators come out
            # pre-divided by CA (normalised at then carries a factor CA).
            nc.vector.memset(vaug[:, :, :, HD], 1.0 / CA)
            nc.vector.memset(vaug8[:, :, :, :, HD], 1.0 / CA)

            def ka():
                # HAM keep-alive: a 32-col dummy weight load (~27ns) resets
                # the PE idle timer so sparse phases don't get clock-gated.
                nc.tensor.ldweights(tri_sb[:, 0:32])

            def norm_unit(i, p, bpsum):
                """Broadcast 1/denominator and write normalised at/at8."""
                ssl = slice(i * SI, (i + 1) * SI)
                bc = bpsum.tile([128, SI], F32, tag="bc", name="bc")
                for l in range(2):
                    nc.tensor.matmul(
                        bc[64 * l:64 * l + 64, :],
                        ones_sb[64:65, 0:64],
                        rden[64:65, p, i, l, :],
                        start=True,
                        stop=True,
                    )
                ka()
                rbc = copool.tile([128, SI], F32, tag="rbc", name="rbc")
                nc.vector.reciprocal_approx_fast(rbc[:, :], bc[:, :])
                if i == 0:
                    nc.vector.tensor_mul(at[:, p, ssl], at[:, p, ssl], rbc[:, :])
                else:
                    nc.vector.tensor_mul(
                        at8[:, p // 2, p % 2, (i - 1) * SI:i * SI],
                        at[:, p, ssl],
                        rbc[:, :],
                    )

            def wo_chain(i, mt, cpsum):
                """Output projection for one 128-row block of out, stripe i."""
                ssl = slice(i * SI, (i + 1) * SI)
                ps = cpsum.tile([128, SI], F32, tag="cps", name="cps")
                if i == 0:
                    for kt in range(4):
                        nc.tensor.matmul(
                            ps[:, :],
                            wo_bf[:, kt, mt * 128:(mt + 1) * 128],
                            at[:, kt, ssl],
                            start=(kt == 0),
                            stop=(kt == 3),
                        )
                    oscale = 1.0 / CA
                else:
                    a8sl = slice((i - 1) * SI, i * SI)
                    for ktp in range(2):
                        nc.tensor.matmul(
                            ps[:, :],
                            wo8_sb[:, ktp, :, mt * 128:(mt + 1) * 128],
                            at8[:, ktp, :, a8sl],
                            start=(ktp == 0),
                            stop=(ktp == 1),
                            perf_mode=DR,
                        )
                    oscale = 1.0 / (CA * CO)
                osb = copool.tile([128, SI], F32, tag="osb", name="osb")
                nc.vector.tensor_scalar_mul(osb[:, :], ps[:, :], oscale)
                nc.sync.dma_start(out=out_v[:, mt, ssl], in_=osb[:, :])
                ka()

            with (
                tc.tile_pool(name="ptb", bufs=4) as ptbpool,
                tc.tile_pool(name="pt8", bufs=4) as pt8pool,
                tc.tile_pool(name="stps", bufs=2, space=MemorySpace.PSUM) as stps,
                tc.tile_pool(name="ops", bufs=2, space=MemorySpace.PSUM) as ops,
            ):

                def make_stripe(i, pair_done=None):
                    """Emission units for attention stripe i (all pairs).

                    Unit = one key-block-pair jp: 4 QK matmuls (row-tiled
                    concurrent head pairs), one big exp ACT, causal masks on
                    GpSimd for diagonal blocks, then PV (fp8 DoubleRow for
                    off-diagonal jp, bf16 for the diagonal ones).
                    """
                    si0 = i * SI
                    ssl = slice(si0, si0 + SI)
                    njs = 4 * i + 4
                    units = []
                    for p in range(NPAIR):
                        state = {}

                        def start_pair(p=p, state=state):
                            state["oa"] = ops.tile(
                                [65, SI], F32, tag="o2", name="oa"
                            )
                            state["ob"] = ops.tile(
                                [65, SI], F32, tag="o2", name="ob"
                            )

                        def unit2(j0, p=p, state=state, i=i, si0=si0):
                            """One emission unit = key blocks (j0, j0+1):
                            both QK pairs back-to-back (one 64-mode window),
                            both exps, masks, then the PV matmuls."""
                            njs = 4 * i + 4
                            nondiag = j0 < 4 * i
                            sts, pts, r0s = [], [], []
                            for j in (j0, j0 + 1):
                                d0 = j * SJ - si0
                                r0 = max(0, d0)
                                r0s.append(r0)
                                st = stps.tile(
                                    [128, 2, SI], F32, tag="st", name="st"
                                )
                                sts.append(st)
                                for l in range(2):
                                    lsl = slice(64 * l, 64 * l + 64)
                                    nc.tensor.matmul(
                                        st[:, l, r0:SI],
                                        kT[lsl, p, j * SJ:(j + 1) * SJ],
                                        qT[lsl, p, si0 + r0:si0 + SI],
                                        start=True,
                                        stop=True,
                                    )
                            if nondiag:
                                pt8 = pt8pool.tile(
                                    [128, 2, 2, SI], FP8, tag="pt8", name="pt8"
                                )
                                for jj in range(2):
                                    nc.scalar.activation(
                                        pt8[:, jj, :, :], sts[jj][:, :, :], AF.Exp
                                    )
                                for l in range(2):
                                    nc.tensor.matmul(
                                        (state["oa"] if l == 0 else state["ob"])[:, :],
                                        vaug8[:, j0 // 2, :, 2 * p + l, 0:HD + 1],
                                        pt8[:, :, l, :],
                                        start=(j0 == 0),
                                        stop=False,
                                        perf_mode=DR,
                                        skip_group_check=True,
                                    )
                            else:
                                for jj in range(2):
                                    j = j0 + jj
                                    r0 = r0s[jj]
                                    d0 = j * SJ - si0
                                    pt = ptbpool.tile(
                                        [128, 2, SI], BF16, tag="ptb", name="ptb"
                                    )
                                    pts.append(pt)
                                    nc.scalar.activation(
                                        pt[:, :, r0:SI], sts[jj][:, :, r0:SI], AF.Exp
                                    )
                                    if d0 >= 0:
                                        for l in range(2):
                                            nc.vector.tensor_mul(
                                                pt[:, l, d0:d0 + 128],
                                                pt[:, l, d0:d0 + 128],
                                                tri_sb[:, :],
                                            )
                                for jj in range(2):
                                    j = j0 + jj
                                    r0 = r0s[jj]
                                    for l in range(2):
                                        nc.tensor.matmul(
                                            (state["oa"] if l == 0 else state["ob"])[:, r0:SI],
                                            vaug[:, j, 2 * p + l, :],
                                            pts[jj][:, l, r0:SI],
                                            start=(j == 0),
                                            stop=(j == njs - 1),
                                            skip_group_check=True,
                                        )

                        def unit2_ka(j0, u=None, i=i):
                            u(j0)
                            npulse = 2 if i < 2 else 4
                            for _ in range(npulse):
                                ka()

                        def end_pair(p=p, state=state, ssl=ssl, i=i):
                            ka()
                            for l in range(2):
                                o2 = state["oa"] if l == 0 else state["ob"]
                                nc.vector.tensor_copy(
                                    at[64 * l:64 * l + 64, p, ssl], o2[0:HD, :]
                                )
                                nc.vector.tensor_copy(
                                    rden[64:65, p, i, l, :], o2[HD:HD + 1, :]
                                )
                            ka()

                        units.append(start_pair)
                        for j0 in range(0, njs, 2):
                            units.append(lambda j0=j0, u=unit2: unit2_ka(j0, u))
                        units.append(end_pair)
                        if pair_done is not None:
                            units.append(lambda p=p: pair_done(p))
                    return units

                def emit_interleaved(units, fillers):
                    """Emit units with fillers distributed evenly between."""
                    U, F = len(units), len(fillers)
                    fi = 0
                    for k, u in enumerate(units):
                        u()
                        want = (k + 1) * F // U
                        while fi < want:
                            fillers[fi]()
                            fi += 1
                    while fi < F:
                        fillers[fi]()
                        fi += 1

                with (
                    tc.tile_pool(name="ax", bufs=3) as xpool,
                    tc.tile_pool(name="ax0", bufs=2) as x0pool,
                    tc.tile_pool(name="aw", bufs=1) as wpool,
                    tc.tile_pool(name="aps", bufs=1, space=MemorySpace.PSUM) as apsum,
                ):
                    wq_sb = wpool.tile([128, KC, DG], BF16, tag="w0")
                    wk_sb = wpool.tile([128, KC, DG], BF16, tag="wk")
                    wv_sb = wpool.tile([128, KC, DG], BF16, tag="w0")
                    w8q_sb = wpool.tile([128, 4, 2, DG], FP8, tag="w8q")
                    w8k_sb = wpool.tile([128, 4, 2, DG], FP8, tag="w8k")
                    w8v_sb = wpool.tile([128, 4, 2, DG], FP8, tag="w8v")
                    w8_sb = {"q": w8q_sb, "k": w8k_sb, "v": w8v_sb}
                    x8_view = {"q": x8q_v, "k": x8k_v, "v": x8v_v}
                    x8_tiles = {}
                    pscale = {"q": 1.0 / CQ, "k": 1.0 / CK, "v": 1.0 / CV}

                    def dma_x8(t, n):
                        xt = xpool.tile(
                            [128, 4, 2, AC], FP8, tag="x8", name=f"x8_{t}{n}"
                        )
                        nc.sync.dma_start(
                            out=xt[:], in_=x8_view[t][:, :, :, n * AC:(n + 1) * AC]
                        )
                        x8_tiles[(t, n)] = xt

                    def chain_qk8(t, n, p):
                        """fp8 DoubleRow q/k projection chain, chunks 1-3."""
                        sl = slice(n * AC, (n + 1) * AC)
                        xt = x8_tiles[(t, n)]
                        ps = apsum.tile([128, AC], F32, tag="aps", name="aps")
                        for kcp in range(4):
                            nc.tensor.matmul(
                                ps[:, :],
                                w8_sb[t][:, kcp, :, p * 128:(p + 1) * 128],
                                xt[:, kcp, :, :],
                                start=(kcp == 0),
                                stop=(kcp == 3),
                                perf_mode=DR,
                            )
                        nc.vector.tensor_scalar_mul(
                            (qT if t == "q" else kT)[:, p, sl], ps[:, :], pscale[t]
                        )

                    def chain_v8(n, mm):
                        """fp8 DoubleRow v projection, one 128-seq block."""
                        xt = x8_tiles[("v", n)]
                        j = n * (AC // 128) + mm
                        ps = apsum.tile([128, DG], F32, tag="aps", name="apsv")
                        for kcp in range(4):
                            nc.tensor.matmul(
                                ps[:, :],
                                xt[:, kcp, :, mm * 128:(mm + 1) * 128],
                                w8_sb["v"][:, kcp, :, :],
                                start=(kcp == 0),
                                stop=(kcp == 3),
                                perf_mode=DR,
                            )
                        nc.vector.tensor_scalar_mul(
                            vaug[:, j, :, 0:HD], ps[:, :], pscale["v"]
                        )
                        if j < 12:
                            nc.vector.tensor_scalar_mul(
                                vaug8[:, j // 2, j % 2, :, 0:HD],
                                ps[:, :],
                                pscale["v"],
                            )

                    def chunk_fillers(n):
                        fs = [lambda t=t, n=n: dma_x8(t, n) for t in ("q", "k", "v")]
                        for p in range(NPAIR):
                            fs.append(lambda p=p, n=n: chain_qk8("q", n, p))
                        for p in range(NPAIR):
                            fs.append(lambda p=p, n=n: chain_qk8("k", n, p))
                        for mm in range(AC // 128):
                            fs.append(lambda mm=mm, n=n: chain_v8(n, mm))
                        return fs

                    # chunk 0: stream the q projection kc-major behind
                    # per-kc DMA slices so the first matmul fires after the
                    # first 128KB lands instead of the full 0.5MB.
                    xt0 = x0pool.tile([128, KC, AC], BF16, tag="x0", name="x_q0")
                    for kc in range(KC):
                        nc.sync.dma_start(
                            out=wq_sb[:, kc, :], in_=wq_v[:, kc, :]
                        )
                        nc.sync.dma_start(
                            out=xt0[:, kc, :], in_=xq0_v[:, kc, :]
                        )
                    # HAM warmup: ~4.5us of throwaway matmuls gated on the
                    # tiny tri/ones DMAs so they run while the first input
                    # slices land and hand over to the q0 chains with no gap.
                    warm = apsum.tile([64, 128], F32, tag="aps", name="warm")
                    for _ in range(44):
                        nc.tensor.matmul(
                            warm[:, :], ones_sb[:, :], tri_sb[:, :],
                            start=True, stop=True,
                        )
                    for ph in range(2):
                        pss = [
                            apsum.tile([128, AC], F32, tag="q0ps", name="q0ps")
                            for _ in range(2)
                        ]
                        for kc in range(KC):
                            for pi in range(2):
                                p = 2 * ph + pi
                                nc.tensor.matmul(
                                    pss[pi][:, :],
                                    wq_sb[:, kc, p * 128:(p + 1) * 128],
                                    xt0[:, kc, :],
                                    start=(kc == 0),
                                    stop=(kc == KC - 1),
                                )
                        for pi in range(2):
                            nc.vector.tensor_copy(
                                qT[:, 2 * ph + pi, 0:AC], pss[pi][:, :]
                            )
                    # chunk-0 k and v (bf16)
                    xtk0 = x0pool.tile([128, KC, AC], BF16, tag="x0", name="x_k0")
                    nc.sync.dma_start(out=xtk0[:], in_=xk0_v[:, :, :])
                    nc.sync.dma_start(out=wk_sb[:], in_=wk_v[:, :, :])
                    nc.sync.dma_start(out=wv_sb[:], in_=wv_v[:, :, :])
                    xtv0 = x0pool.tile([128, KC, AC], BF16, tag="x0", name="x_v0")
                    nc.sync.dma_start(out=xtv0[:], in_=xv0_v[:, :, :])
                    nc.sync.dma_start(out=w8q_sb[:], in_=w8q_v[:, :, :, :])
                    nc.sync.dma_start(out=w8k_sb[:], in_=w8k_v[:, :, :, :])
                    nc.sync.dma_start(out=w8v_sb[:], in_=w8v_v[:, :, :, :])

                    def chain_k0(p):
                        ps = apsum.tile([128, AC], F32, tag="aps", name="apsk0")
                        for kc in range(KC):
                            nc.tensor.matmul(
                                ps[:, :],
                                wk_sb[:, kc, p * 128:(p + 1) * 128],
                                xtk0[:, kc, :],
                                start=(kc == 0),
                                stop=(kc == KC - 1),
                            )
                        nc.vector.tensor_copy(kT[:, p, 0:AC], ps[:, :])

                    def chain_v0(mm):
                        ps = apsum.tile([128, DG], F32, tag="aps", name="apsv0")
                        for kc in range(KC):
                            nc.tensor.matmul(
                                ps[:, :],
                                xtv0[:, kc, mm * 128:(mm + 1) * 128],
                                wv_sb[:, kc, :],
                                start=(kc == 0),
                                stop=(kc == KC - 1),
                            )
                        nc.vector.tensor_copy(vaug[:, mm, :, 0:HD], ps[:, :])
                        nc.vector.tensor_copy(
                            vaug8[:, mm // 2, mm % 2, :, 0:HD], ps[:, :]
                        )

                    for p in range(NPAIR):
                        chain_k0(p)
                    for mm in range(AC // 128):
                        chain_v0(mm)

                    emit_interleaved(make_stripe(0), chunk_fillers(1))
                    emit_interleaved(make_stripe(1), chunk_fillers(2))
                    emit_interleaved(make_stripe(2), chunk_fillers(3))

                # ---- last stripe interleaves with normalise + Wo (1-buf
                # ---- psum pools; stalls absorb into exp waits)
                with (
                    tc.tile_pool(name="cps1", bufs=1, space=MemorySpace.PSUM) as cps1,
                    tc.tile_pool(name="bps1", bufs=1, space=MemorySpace.PSUM) as bps1,
                ):
                    c_fillers = [
                        lambda: nc.sync.dma_start(out=wo_bf[:], in_=wo_v[:, :, :]),
                        lambda: nc.sync.dma_start(out=wo8_sb[:], in_=wo8_v[:, :, :, :]),
                    ]
                    for i in range(3):
                        for p in range(NPAIR):
                            c_fillers.append(
                                lambda i=i, p=p: norm_unit(i, p, bps1)
                            )
                        for mt in range(8):
                            c_fillers.append(
                                lambda i=i, mt=mt: wo_chain(i, mt, cps1)
                            )

                    emit_interleaved(
                        make_stripe(3, pair_done=lambda p: norm_unit(3, p, bps1)),
                        c_fillers,
                    )

            # final output stripe with room to pipeline (st/o2 closed)
            with (
                tc.tile_pool(name="cps2", bufs=4, space=MemorySpace.PSUM) as cps2,
            ):
                for mt in range(8):
                    wo_chain(3, mt, cps2)

    nc.compile()
    return nc


_NC_CACHE = {}


def _get_nc():
    if "nc" not in _NC_CACHE:
        _NC_CACHE["nc"] = _build_core_kernel()
    return _NC_CACHE["nc"]


def make_in_maps(query, key, value, Wq, Wk, Wv, Wo):
    import ml_dtypes

    bf = ml_dtypes.bfloat16
    f8 = ml_dtypes.float8_e4m3
    tri = (np.arange(128)[None, :] >= np.arange(128)[:, None]).astype(bf)
    ones128 = np.ones((128, 64), bf)
    # per-batch transposed activations (shared by the batch's two cores):
    # chunk 0 in bf16, the full sequence in fp8
    xq0_b, xk0_b, xv0_b, x8q_b, x8k_b, x8v_b = [], [], [], [], [], []
    for b in range(B):
        for src, lo, hi in ((query, xq0_b, x8q_b), (key, xk0_b, x8k_b),
                            (value, xv0_b, x8v_b)):
            xt = np.ascontiguousarray(src[b].T)
            lo.append(xt[:, :AC].astype(bf))
            hi.append(xt.astype(f8))
    w_g = []
    for g in range(2):
        rows = slice(g * DG, (g + 1) * DG)
        wq_g = Wq[rows, :] * SCALE
        w_g.append({
            "wq": np.ascontiguousarray(wq_g.T).astype(bf),
            "wk": np.ascontiguousarray(Wk[rows, :].T).astype(bf),
            "wv": np.ascontiguousarray(Wv[rows, :].T).astype(bf),
            "w8q": np.ascontiguousarray((wq_g * CQ).T).astype(f8),
            "w8k": np.ascontiguousarray((Wk[rows, :] * CK).T).astype(f8),
            "w8v": np.ascontiguousarray((Wv[rows, :] * CV).T).astype(f8),
            "wo": np.ascontiguousarray(Wo[:, rows].T).astype(bf),
            "wo8": np.ascontiguousarray((Wo[:, rows] * CO).T).astype(f8),
        })
    in_maps = []
    for c in range(NCORES):
        b, g = c // 2, c % 2
        in_maps.append({
            "xq0": xq0_b[b], "xk0": xk0_b[b], "xv0": xv0_b[b],
            "x8q": x8q_b[b], "x8k": x8k_b[b], "x8v": x8v_b[b],
            **w_g[g], "tri": tri, "ones128": ones128,
        })
    return in_maps


def kernel(query, key, value, attn_mask, Wq, Wk, Wv, Wo):
    global LAST_RESULTS
    from concourse.bass_utils import run_bass_kernel_spmd

    query = np.asarray(query, np.float32)
    key = np.asarray(key, np.float32)
    value = np.asarray(value, np.float32)
    Wq = np.asarray(Wq, np.float32)
    Wk = np.asarray(Wk, np.float32)
    Wv = np.asarray(Wv, np.float32)
    Wo = np.asarray(Wo, np.float32)

    nc = _get_nc()
    in_maps = make_in_maps(query, key, value, Wq, Wk, Wv, Wo)
    res = run_bass_kernel_spmd(
        nc,
        in_maps,
        core_ids=list(range(NCORES)),
        trace=bool(int(os.environ.get("KERNEL_TRACE", "0"))),
    )
    LAST_RESULTS = res

    full = np.empty((B, S, DIM), np.float32)
    for b in range(B):
        full[b] = (res.results[2 * b]["out"] + res.results[2 * b + 1]["out"]).T
    return full
